# revision 10
# baseline (speedup 1.0000x reference)
"""MultiHeadAttention (B=4, S=2048, d_model=1024, H=16, dh=64) on 8 trn2 cores.

Sharding: core (b, g) = batch b in 0..3, head-group g in 0..1 (8 heads each).
Each core computes, for its (b, g):
  Q^T, K^T  [512, 2048] head-dim-major; V [2048, 512] token-major (+ ones col)
  transposed scores S^T = K^T_tile.T @ Q^T per (head, k-tile 128, q-tile 512)
  P = exp(S^T / 8) (no max subtraction; scores are O(1)); causal masking via
  affine_select (skip fully-masked k-tiles entirely)
  fused AV+rowsum: lhsT = [V | 1] -> psum [65, 512]; ctx normalized by 1/l via
  gpsimd partition_broadcast + one tensor_tensor (PSUM operand)
  partial output projection y_partial = ctx^T.T @ wo[:, group].T
Host sums the two groups' partials per batch and adds bo.

All matmul operands are bfloat16 (full-rate PE, half the DMA/LDWEIGHTS
bytes of fp32r; PSUM accumulation stays fp32).

Schedule: the PE queue is in-order, so the AV matmuls for k-tile kk are
emitted one slot late (after the scores for kk+1 and an interleave quantum)
so the exp (scalar engine, ~1us) and affine_select (gpsimd) latencies hide
behind queued PE work.  Straddle (diagonal) tiles trim the q range of the
scores matmul, exp, mask and AV to [t_off:512] with t_off = min(qoff, 256)
(fp32r matmuls need a moving dim >= 256 for full rate).
"""
import sys
sys.path.insert(0, "/opt/trn_rl_repo")

import numpy as np

import concourse.bass as bass
import concourse.mybir as mybir
import concourse.tile as tile
from concourse import bacc
from concourse.bass_utils import run_bass_kernel_spmd

F32 = mybir.dt.float32
F32R = mybir.dt.float32r
BF16 = mybir.dt.bfloat16
AF = mybir.ActivationFunctionType
AL = mybir.AluOpType

B, S, D, H, DH = 4, 2048, 1024, 16, 64
NC = 8
G = 2              # head groups (cores per batch)
HPC = H // G       # 8 heads per core
EH = HPC * DH      # 512
NQT = S // 512     # 4 q-tiles
NKT = S // 128     # 16 k-tiles
NDK = D // 128     # 8 contraction subtiles
SCALE = 1.0 / np.sqrt(DH)

_cache = {}
MM_NAMES = {"sc0": set(), "sc1": set(), "av": set(), "pj": set(), "p3": set()}


def _build(mode, k_needed, k_full, mixed):
    """Build the per-core Bass program.

    mode: "affine" (causal / prefix masks expressible as q-k>=const... strictly
          the tril case) or "dense" (per-element 0/1 mask multiply from DRAM).
    k_needed[qt]: number of leading k-tiles to compute for q-tile qt.
    k_full[qt]:   k-tiles below this index need no masking.
    mixed: set of (qt, kk) needing a mask op (affine: affine_select;
           dense: sel-tile multiply).
    """
    nc = bacc.Bacc("TRN2", target_bir_lowering=False, debug=False, num_devices=NC)

    xqT_d = nc.dram_tensor("xqT", [D, S], BF16, kind="ExternalInput").ap()
    xkT_d = nc.dram_tensor("xkT", [D, S], BF16, kind="ExternalInput").ap()
    xvT_d = nc.dram_tensor("xvT", [D, S], BF16, kind="ExternalInput").ap()
    wqT_d = nc.dram_tensor("wqT", [128, NDK, EH], BF16, kind="ExternalInput").ap()
    wkT_d = nc.dram_tensor("wkT", [128, NDK, EH], BF16, kind="ExternalInput").ap()
    wvT_d = nc.dram_tensor("wvT", [128, NDK, EH], BF16, kind="ExternalInput").ap()
    bq_d = nc.dram_tensor("bq", [128, 4], F32, kind="ExternalInput").ap()
    bk_d = nc.dram_tensor("bk", [128, 4], F32, kind="ExternalInput").ap()
    bv_d = nc.dram_tensor("bv", [128, EH], F32, kind="ExternalInput").ap()
    woT_d = nc.dram_tensor("woT", [128, 4, D], BF16, kind="ExternalInput").ap()
    ones_d = nc.dram_tensor("ones1", [128, 1], BF16, kind="ExternalInput").ap()
    if mode == "dense":
        mT_d = nc.dram_tensor("maskT", [S, S], BF16, kind="ExternalInput").ap()
        mT_v = mT_d.rearrange("(kt p) q -> p kt q", p=128)
    y_d = nc.dram_tensor("y", [S, D], BF16, kind="ExternalOutput").ap()

    xq_v = xqT_d.rearrange("(dk p) t -> p dk t", p=128)
    xk_v = xkT_d.rearrange("(dk p) t -> p dk t", p=128)
    xv_v = xvT_d.rearrange("(dk p) t -> p dk t", p=128)

    with tile.TileContext(nc) as tc:
        with nc.allow_low_precision(reason="fp32r storage has fp32 width"):
            _body(nc, tc, mode, k_needed, k_full, mixed,
                  xq_v, xk_v, xv_v, wqT_d, wkT_d, wvT_d,
                  bq_d, bk_d, bv_d, woT_d, ones_d,
                  mT_v if mode == "dense" else None, y_d)
    nc.compile()
    return nc


def _body(nc, tc, mode, k_needed, k_full, mixed,
          xq_v, xk_v, xv_v, wqT_d, wkT_d, wvT_d,
          bq_d, bk_d, bv_d, woT_d, ones_d, mT_v, y_d):
    """Interleaved schedule: Q/K/V projection chunks and output-projection
    chunks are emitted *between* attention k-tiles so the PE stays dense
    (HAM warm) and phases overlap.

    Window qt runs attention for all 4 head-pairs on q-tile qt, interleaving:
      window 0: Q/K proj for tq=1, V proj for tv=1
      window 1: tq=2, tv=2, output-proj of q-tile 0
      window 2: tq=3, tv=3, output-proj of q-tile 1
      window 3: output-proj of q-tile 2;   tail: output-proj of q-tile 3
    """
    pers_cm = tc.tile_pool(name="pers", bufs=1)
    pers = pers_cm.__enter__()
    KT = pers.tile([128, 4, S], BF16)            # [part=eh%128, et, t]
    V65 = pers.tile([128, NKT, HPC, 65], BF16)   # [t%128, t//128, h, e|1]
    wo_t = pers.tile([128, 4, D], BF16)
    wv_t = pers.tile([128, NDK, EH], BF16)
    bq_t = pers.tile([128, 4], F32)
    bk_t = pers.tile([128, 4], F32)
    bv_t = pers.tile([128, EH], F32)
    ones_t = pers.tile([128, 1], BF16)

    pw_cm = tc.tile_pool(name="pw", bufs=2)
    pw = pw_cm.__enter__()
    px_cm = tc.tile_pool(name="px", bufs=3)
    px = px_cm.__enter__()
    pq_cm = tc.tile_pool(name="pq", bufs=2)
    pq = pq_cm.__enter__()
    pcw_cm = tc.tile_pool(name="pcw", bufs=3)
    pcw = pcw_cm.__enter__()
    ppt_cm = tc.tile_pool(name="ppt", bufs=3)
    ppt = ppt_cm.__enter__()
    pnrm_cm = tc.tile_pool(name="pnrm", bufs=1)
    pnrm = pnrm_cm.__enter__()
    py_cm = tc.tile_pool(name="py", bufs=2)
    py = py_cm.__enter__()
    pp_cm = tc.tile_pool(name="pp", bufs=2, space="PSUM")
    pp = pp_cm.__enter__()
    psc_cm = tc.tile_pool(name="psc", bufs=2, space="PSUM")
    psc = psc_cm.__enter__()
    pav_cm = tc.tile_pool(name="pav", bufs=1, space="PSUM")
    pav = pav_cm.__enter__()

    qwin = {}    # tq -> [128, 4, 512] Q^T window tile
    ctxw = {}    # qt -> [128, 4, 512] ctx^T window tile
    state = {}   # live w/x tiles for the chunk being emitted

    # ---- chunk closures ----
    def qk_chunks(tq):
        # half-tile loads (dk 0-3 / 4-7) so bufs=3 gives cross-chunk prefetch
        def load(w_d, x_v, kind, half):
            def f():
                hs = slice(half * 4, half * 4 + 4)
                w_t = pw.tile([128, 4, EH], BF16, tag="w",
                              name=f"w_{kind}{tq}{half}")
                nc.sync.dma_start(w_t[:], w_d[:, hs, :])
                x_t = px.tile([128, 4, 512], BF16, tag="x",
                              name=f"x_{kind}{tq}{half}")
                nc.sync.dma_start(x_t[:], x_v[:, hs, tq * 512:(tq + 1) * 512])
                state[f"w{half}"], state[f"x{half}"] = w_t, x_t
                if kind == "q" and half == 0:
                    qwin[tq] = pq.tile([128, 4, 512], BF16, tag="qw",
                                       name=f"qw{tq}")
            return f

        def mmgroup(et, kind):
            def f():
                ps_t = pp.tile([128, 512], F32, tag="pj", name=f"ps_{kind}{tq}_{et}")
                for dk in range(NDK):
                    w_t = state[f"w{dk // 4}"]
                    x_t = state[f"x{dk // 4}"]
                    mi = nc.tensor.matmul(ps_t[:],
                                     w_t[:, dk % 4, et * 128:(et + 1) * 128],
                                     x_t[:, dk % 4, :],
                                     start=(dk == 0), stop=(dk == NDK - 1))
                    MM_NAMES["pj"].add(mi.ins.name)
                if kind == "q":
                    nc.vector.tensor_tensor(
                        qwin[tq][:, et, :], ps_t[:],
                        bq_t[:, et:et + 1].to_broadcast([128, 512]), AL.add)
                else:
                    nc.vector.tensor_tensor(
                        KT[:, et, tq * 512:(tq + 1) * 512], ps_t[:],
                        bk_t[:, et:et + 1].to_broadcast([128, 512]), AL.add)
            return f

        out = []
        for kind, w_d, x_v in (("q", wqT_d, xq_v), ("k", wkT_d, xk_v)):
            for half in (0, 1):
                g = load(w_d, x_v, kind, half)
                g.mms = 0
                out.append(g)
            for et in range(4):
                g = mmgroup(et, kind)
                g.mms = 8
                out.append(g)
        return out

    def v_chunks(tv):
        def load(half):
            def f():
                hs = slice(half * 4, half * 4 + 4)
                x_t = px.tile([128, 4, 512], BF16, tag="x", name=f"x_v{tv}{half}")
                nc.sync.dma_start(x_t[:], xv_v[:, hs, tv * 512:(tv + 1) * 512])
                state[f"x{half}"] = x_t
            return f

        def mmgroup(tl):
            def f():
                tt = tv * 4 + tl
                ps_t = pp.tile([128, 512], F32, tag="pj", name=f"ps_v{tt}")
                for dk in range(NDK):
                    x_t = state[f"x{dk // 4}"]
                    mi = nc.tensor.matmul(ps_t[:],
                                     x_t[:, dk % 4, tl * 128:(tl + 1) * 128],
                                     wv_t[:, dk, :],
                                     start=(dk == 0), stop=(dk == NDK - 1))
                    MM_NAMES["pj"].add(mi.ins.name)
                nc.vector.tensor_tensor(
                    V65[:, tt, :, 0:64],
                    ps_t.rearrange("p (h e) -> p h e", h=HPC),
                    bv_t.rearrange("p (h e) -> p h e", h=HPC), AL.add)
            return f

        out = []
        for half in (0, 1):
            g = load(half)
            g.mms = 0
            out.append(g)
        for tl in range(4):
            g = mmgroup(tl)
            g.mms = 8
            out.append(g)
        return out

    def p3_chunks(qt):
        p3state = {}

        def half_a(tl, mc):
            def f():
                cw = ctxw[qt]
                tt = qt * 4 + tl
                ps_t = pp.tile([128, 512], F32, tag="pj", name=f"ps_o{tt}_{mc}")
                p3state[(tl, mc)] = ps_t
                for hp in range(2):
                    mi = nc.tensor.matmul(ps_t[:],
                                          cw[:, hp, tl * 128:(tl + 1) * 128],
                                          wo_t[:, hp, mc * 512:(mc + 1) * 512],
                                          start=(hp == 0), stop=False)
                    MM_NAMES["p3"].add(mi.ins.name)
            f.mms = 2
            return f

        def half_b(tl, mc):
            def f():
                cw = ctxw[qt]
                tt = qt * 4 + tl
                ps_t = p3state.pop((tl, mc))
                for hp in range(2, 4):
                    mi = nc.tensor.matmul(ps_t[:],
                                          cw[:, hp, tl * 128:(tl + 1) * 128],
                                          wo_t[:, hp, mc * 512:(mc + 1) * 512],
                                          start=False, stop=(hp == 3))
                    MM_NAMES["p3"].add(mi.ins.name)
                y_t = py.tile([128, 512], BF16, tag="y", name=f"y{tt}_{mc}")
                nc.vector.tensor_copy(y_t[:], ps_t[:])
                nc.sync.dma_start(
                    y_d[tt * 128:(tt + 1) * 128, mc * 512:(mc + 1) * 512],
                    y_t[:])
            f.mms = 2
            return f

        out = []
        for tl in range(4):
            for mc in range(2):
                out.append(half_a(tl, mc))
                out.append(half_b(tl, mc))
        return out

    # ---- attention window with interleaved work ----
    def window(qt, work):
        klim = k_needed[qt]
        q0 = qt * 512
        cw = pcw.tile([128, 4, 512], BF16, tag="cw", name=f"cw{qt}")
        ctxw[qt] = cw
        qw = qwin[qt]
        n_tiles = 4 * klim
        total_mms = sum(getattr(f, "mms", 4) for f in work) or 1
        wi = 0
        emitted = 0
        done = 0

        av_tiles = {}   # hp -> (av0, av1)

        def emit_scores(hp, kk):
            """scores matmul pair + exp (+ mask) for (hp, kk); returns the
            closure that emits the delayed AV matmuls."""
            straddle = (qt, kk) in mixed
            qoff = max(0, kk * 128 - q0) if (straddle and mode == "affine") else 0
            moff = 128 if qoff >= 128 else 0
            s_t = psc.tile([128, 2, 512], F32, tag="sc")
            for j in range(2):
                mi = nc.tensor.matmul(
                    s_t[:, j, moff:512],
                    KT[j * 64:(j + 1) * 64, hp, kk * 128:(kk + 1) * 128],
                    qw[j * 64:(j + 1) * 64, hp, moff:512],
                    start=True, stop=True, tile_position=(j * 64, 0))
                MM_NAMES[f"sc{j}"].add(mi.ins.name)
            p_t = ppt.tile([128, 2, 512], BF16, tag="pt")
            nc.scalar.activation(p_t[:, :, qoff:512], s_t[:, :, qoff:512],
                                 AF.Exp, scale=float(SCALE))
            if straddle:
                if mode == "affine":
                    nc.gpsimd.affine_select(
                        p_t[:], p_t[:], pattern=[[0, 2], [1, 512]],
                        compare_op=AL.is_ge, fill=0.0,
                        base=q0 - kk * 128, channel_multiplier=-1)
                else:
                    sel_t = ppt.tile([128, 512], BF16, tag="sel")
                    nc.sync.dma_start(sel_t[:], mT_v[:, kk, q0:q0 + 512])
                    nc.vector.tensor_tensor(
                        p_t[:], p_t[:],
                        sel_t[:, None, :].to_broadcast([128, 2, 512]),
                        AL.mult)

            def emit_av():
                if kk == 0:
                    # allocate at first use: allocating earlier would put the
                    # pool release boundary (bufs=1, aliases the previous
                    # hp's accumulators) before the previous hp's final AV
                    # and norm reads - a race.
                    av_tiles[hp] = (
                        pav.tile([65, 512], F32, tag="av0",
                                 name=f"av0_{qt}_{hp}"),
                        pav.tile([65, 512], F32, tag="av1",
                                 name=f"av1_{qt}_{hp}"))
                av0, av1 = av_tiles[hp]
                for j, av in ((0, av0), (1, av1)):
                    mi = nc.tensor.matmul(av[:], V65[:, kk, 2 * hp + j, :],
                                     p_t[:, j, :],
                                     start=(kk == 0), stop=(kk == klim - 1))
                    MM_NAMES["av"].add(mi.ins.name)
            return emit_av

        def emit_norm(hp):
            # copy to SBUF first so the psum accumulators free early;
            # head1 lands at partitions 64-127 to keep TT bases aligned
            av0, av1 = av_tiles.pop(hp)
            avc = pnrm.tile([128, 512], F32, tag="avc", bufs=1)
            lin = pnrm.tile([1, 2, 512], F32, tag="lin")
            nc.vector.tensor_copy(avc[0:64, :], av0[0:64, :])
            nc.vector.tensor_copy(avc[64:128, :], av1[0:64, :])
            nc.vector.tensor_copy(lin[:, 0, :], av0[64:65, :])
            nc.vector.tensor_copy(lin[:, 1, :], av1[64:65, :])
            lrec = pnrm.tile([1, 2, 512], F32, tag="lrec")
            rec_bc = pnrm.tile([128, 2, 512], F32, tag="rbc")
            # rec_bc[0:1] doubles as reciprocal scratch (pbcast overwrites it)
            nc.vector.reciprocal_approx_accurate(lrec[:], lin[:],
                                                 rec_bc[0:1, :, :])
            nc.gpsimd.partition_broadcast(rec_bc[:], lrec[0:1, :, :])
            nc.vector.tensor_tensor(cw[0:64, hp, :],
                                    avc[0:64, :], rec_bc[0:64, 0, :], AL.mult)
            nc.vector.tensor_tensor(cw[64:128, hp, :],
                                    avc[64:128, :], rec_bc[64:128, 1, :], AL.mult)

        # flat software pipeline over (hp, kk): AV for slot i is emitted
        # after the scores for slot i+1 plus an interleave quantum, so the
        # exp/mask latency hides behind queued PE work.
        slots = [(hp, kk) for hp in range(4) for kk in range(klim)]
        pending_av = None
        pending_hp = None
        for hp, kk in slots:
            av_f = emit_scores(hp, kk)
            done += 1
            target = done * total_mms / n_tiles
            while wi < len(work) and emitted < target:
                emitted += getattr(work[wi], "mms", 4)
                work[wi]()
                wi += 1
            if pending_av is not None:
                pending_av()
                if pending_hp is not None:
                    emit_norm(pending_hp)
                    pending_hp = None
            pending_av = av_f
            if kk == klim - 1:
                pending_hp = hp
        pending_av()
        emit_norm(pending_hp)
        while wi < len(work):
            work[wi]()
            wi += 1

    # ---- prologue: Q/K for tq=0 first (their DMAs gate the first matmul),
    # then the persistent-tile DMAs, then V for tv=0 ----
    for f in qk_chunks(0):
        f()
    nc.sync.dma_start(bq_t[:], bq_d)
    nc.sync.dma_start(bk_t[:], bk_d)
    nc.sync.dma_start(bv_t[:], bv_d)
    nc.sync.dma_start(ones_t[:], ones_d)
    nc.sync.dma_start(wv_t[:], wvT_d)
    nc.vector.tensor_copy(V65[:, :, :, 64:65],
                          ones_t[:, 0:1].to_broadcast([128, NKT, HPC, 1]))
    for f in v_chunks(0):
        f()

    # ---- windows ----
    def wo_load():
        nc.sync.dma_start(wo_t[:], woT_d)
    wo_load.mms = 0

    for qt in range(NQT):
        work = []
        if qt == 0:
            work.append(wo_load)
        if qt + 1 < NQT:
            work += qk_chunks(qt + 1)
            work += v_chunks(qt + 1)
        if qt == 2:
            work += p3_chunks(0)
        elif qt == 3:
            work += p3_chunks(1)
            work += p3_chunks(2)
        window(qt, work)
    for f in p3_chunks(NQT - 1):
        f()

    for cm in (pav_cm, psc_cm, pp_cm, py_cm, pnrm_cm, ppt_cm, pcw_cm, pq_cm,
               px_cm, pw_cm, pers_cm):
        cm.__exit__(None, None, None)


def _analyze_mask(mask):
    """Classify the mask and derive the per-q-tile k-tile structure."""
    m = np.asarray(mask)
    iota = np.arange(S)
    n = m.sum(axis=2)                     # [B, S] count of ones per row
    prefix_ok = bool((m == (iota[None, None, :] < n[..., None])).all())
    causal = prefix_ok and bool((n == iota[None, :] + 1).all())
    allones = bool((m == 1).all())

    k_needed, k_full, mixed = [], [], set()
    if allones:
        mode = "affine"   # no mask ops at all
        k_needed = [NKT] * NQT
        k_full = [NKT] * NQT
    elif causal:
        mode = "affine"
        for qt in range(NQT):
            k_needed.append(4 * qt + 4)
            k_full.append(4 * qt)
            for kk in range(4 * qt, 4 * qt + 4):
                mixed.add((qt, kk))
    else:
        mode = "dense"
        for qt in range(NQT):
            sl = m[:, qt * 512:(qt + 1) * 512, :]       # [B, 512, S]
            need = 0
            full = NKT
            for kk in range(NKT):
                blk = sl[:, :, kk * 128:(kk + 1) * 128]
                if blk.any():
                    need = kk + 1
                if not blk.all():
                    full = min(full, kk)
            need = max(need, 1)
            full = min(full, need)
            k_needed.append(need)
            k_full.append(full)
            for kk in range(full, need):
                blk = sl[:, :, kk * 128:(kk + 1) * 128]
                if not blk.all():
                    mixed.add((qt, kk))
    return mode, tuple(k_needed), tuple(k_full), frozenset(mixed)


def _prep_inputs(x_q, x_k, x_v, mask, wq, wk, wv, bq, bk, bv, wo, mode):
    """Build the 8 per-core input dicts."""
    import ml_dtypes
    f32 = np.float32
    bf16 = ml_dtypes.bfloat16
    in_maps = []
    ones1 = np.ones((128, 1), bf16)
    for core in range(NC):
        b, g = divmod(core, G)
        hs = slice(g * HPC, (g + 1) * HPC)
        im = {
            "xqT": np.ascontiguousarray(np.asarray(x_q[b], f32).T).astype(bf16),
            "xkT": np.ascontiguousarray(np.asarray(x_k[b], f32).T).astype(bf16),
            "xvT": np.ascontiguousarray(np.asarray(x_v[b], f32).T).astype(bf16),
            "ones1": ones1,
        }
        for name, w in (("wqT", wq), ("wkT", wk), ("wvT", wv)):
            # [H, DH, D] group slice -> [D, EH] -> [128, NDK, EH] with d = dk*128+p
            wt = np.asarray(w[hs], f32).transpose(2, 0, 1).reshape(D, EH)
            im[name] = np.ascontiguousarray(wt.reshape(NDK, 128, EH))\
                .transpose(1, 0, 2).astype(bf16)
        for name, bb in (("bq", bq), ("bk", bk)):
            flat = np.asarray(bb[hs], f32).reshape(EH)
            im[name] = np.ascontiguousarray(flat.reshape(4, 128).T)
        im["bv"] = np.broadcast_to(np.asarray(bv[hs], f32).reshape(1, EH),
                                   (128, EH)).copy()
        woT = np.asarray(wo[:, g * EH:(g + 1) * EH], f32).T   # [EH, D]
        im["woT"] = np.ascontiguousarray(woT.reshape(4, 128, D))\
            .transpose(1, 0, 2).astype(bf16)
        if mode == "dense":
            im["maskT"] = np.ascontiguousarray(
                np.asarray(mask[b], f32).T).astype(bf16)
        in_maps.append(im)
    return in_maps


def _run(x_q, x_k, x_v, mask, wq, wk, wv, bq, bk, bv, wo, bo,
         trace=False, trace_cores=None):
    mode, k_needed, k_full, mixed = _analyze_mask(mask)
    key = (mode, k_needed, k_full, mixed)
    if key not in _cache:
        _cache[key] = _build(mode, k_needed, k_full, mixed)
    nc = _cache[key]
    in_maps = _prep_inputs(x_q, x_k, x_v, mask, wq, wk, wv, bq, bk, bv, wo, mode)
    res = run_bass_kernel_spmd(nc, in_maps, core_ids=list(range(NC)),
                               trace=trace, trace_cores=trace_cores)
    bo = np.asarray(bo, np.float32)
    out = np.empty((B, S, D), np.float32)
    for b in range(B):
        out[b] = (res.results[2 * b]["y"].astype(np.float32)
                  + res.results[2 * b + 1]["y"].astype(np.float32) + bo)
    return out, res


def kernel(x_q, x_k, x_v, mask, wq, wk, wv, bq, bk, bv, wo, bo):
    out, _ = _run(x_q, x_k, x_v, mask, wq, wk, wv, bq, bk, bv, wo, bo)
    return out


# revision 11
# speedup vs baseline: 1.1610x; 1.1610x over previous
"""MultiHeadAttention (B=4, S=2048, d_model=1024, H=16, dh=64) on 8 trn2 cores.

Sharding: core (b, g) = batch b in 0..3, head-group g in 0..1 (8 heads each).
Each core computes, for its (b, g):
  Q^T, K^T  [512, 2048] head-dim-major; V [2048, 512] token-major (+ ones col)
  transposed scores S^T = K^T_tile.T @ Q^T per (head, k-tile 128, q-tile 512)
  P = exp(S^T / 8) (no max subtraction; scores are O(1)); causal masking via
  affine_select (skip fully-masked k-tiles entirely)
  fused AV+rowsum: lhsT = [V | 1] -> psum [65, 512]; ctx normalized by 1/l via
  gpsimd partition_broadcast + one tensor_tensor (PSUM operand)
  partial output projection y_partial = ctx^T.T @ wo[:, group].T
Host sums the two groups' partials per batch and adds bo.

All matmul operands are bfloat16 (full-rate PE, half the DMA/LDWEIGHTS
bytes of fp32r; PSUM accumulation stays fp32).

Schedule: the PE queue is in-order, so the AV matmuls for k-tile kk are
emitted one slot late (after the scores for kk+1 and an interleave quantum)
so the exp (scalar engine, ~1us) and affine_select (gpsimd) latencies hide
behind queued PE work.  Straddle (diagonal) tiles trim the q range of the
scores matmul, exp, mask and AV to [t_off:512] with t_off = min(qoff, 256)
(fp32r matmuls need a moving dim >= 256 for full rate).
"""
import sys
sys.path.insert(0, "/opt/trn_rl_repo")

import numpy as np

import concourse.bass as bass
import concourse.mybir as mybir
import concourse.tile as tile
from concourse import bacc
from concourse.bass_utils import run_bass_kernel_spmd

F32 = mybir.dt.float32
F32R = mybir.dt.float32r
BF16 = mybir.dt.bfloat16
AF = mybir.ActivationFunctionType
AL = mybir.AluOpType

B, S, D, H, DH = 4, 2048, 1024, 16, 64
NC = 8
G = 2              # head groups (cores per batch)
HPC = H // G       # 8 heads per core
EH = HPC * DH      # 512
NQT = S // 512     # 4 q-tiles
NKT = S // 128     # 16 k-tiles
NDK = D // 128     # 8 contraction subtiles
SCALE = 1.0 / np.sqrt(DH)

_cache = {}
MM_NAMES = {"sc0": set(), "sc1": set(), "av": set(), "pj": set(), "p3": set()}


def _build(mode, k_needed, k_full, mixed):
    """Build the per-core Bass program.

    mode: "affine" (causal / prefix masks expressible as q-k>=const... strictly
          the tril case) or "dense" (per-element 0/1 mask multiply from DRAM).
    k_needed[qt]: number of leading k-tiles to compute for q-tile qt.
    k_full[qt]:   k-tiles below this index need no masking.
    mixed: set of (qt, kk) needing a mask op (affine: affine_select;
           dense: sel-tile multiply).
    """
    nc = bacc.Bacc("TRN2", target_bir_lowering=False, debug=False, num_devices=NC)

    xqT_d = nc.dram_tensor("xqT", [D, S], BF16, kind="ExternalInput").ap()
    xkT_d = nc.dram_tensor("xkT", [D, S], BF16, kind="ExternalInput").ap()
    xvT_d = nc.dram_tensor("xvT", [D, S], BF16, kind="ExternalInput").ap()
    wqT_d = nc.dram_tensor("wqT", [128, NDK, EH], BF16, kind="ExternalInput").ap()
    wkT_d = nc.dram_tensor("wkT", [128, NDK, EH], BF16, kind="ExternalInput").ap()
    wvT_d = nc.dram_tensor("wvT", [128, NDK, EH], BF16, kind="ExternalInput").ap()
    bq_d = nc.dram_tensor("bq", [128, 4], F32, kind="ExternalInput").ap()
    bk_d = nc.dram_tensor("bk", [128, 4], F32, kind="ExternalInput").ap()
    bv_d = nc.dram_tensor("bv", [128, EH], F32, kind="ExternalInput").ap()
    woT_d = nc.dram_tensor("woT", [128, 4, D], BF16, kind="ExternalInput").ap()
    ones_d = nc.dram_tensor("ones1", [128, 1], BF16, kind="ExternalInput").ap()
    if mode == "dense":
        mT_d = nc.dram_tensor("maskT", [S, S], BF16, kind="ExternalInput").ap()
        mT_v = mT_d.rearrange("(kt p) q -> p kt q", p=128)
    y_d = nc.dram_tensor("y", [S, D], BF16, kind="ExternalOutput").ap()

    xq_v = xqT_d.rearrange("(dk p) t -> p dk t", p=128)
    xk_v = xkT_d.rearrange("(dk p) t -> p dk t", p=128)
    xv_v = xvT_d.rearrange("(dk p) t -> p dk t", p=128)

    with tile.TileContext(nc) as tc:
        with nc.allow_low_precision(reason="fp32r storage has fp32 width"):
            _body(nc, tc, mode, k_needed, k_full, mixed,
                  xq_v, xk_v, xv_v, wqT_d, wkT_d, wvT_d,
                  bq_d, bk_d, bv_d, woT_d, ones_d,
                  mT_v if mode == "dense" else None, y_d)
    nc.compile()
    return nc


def _body(nc, tc, mode, k_needed, k_full, mixed,
          xq_v, xk_v, xv_v, wqT_d, wkT_d, wvT_d,
          bq_d, bk_d, bv_d, woT_d, ones_d, mT_v, y_d):
    """Interleaved schedule: Q/K/V projection chunks and output-projection
    chunks are emitted *between* attention k-tiles so the PE stays dense
    (HAM warm) and phases overlap.

    Window qt runs attention for all 4 head-pairs on q-tile qt, interleaving:
      window 0: Q/K proj for tq=1, V proj for tv=1
      window 1: tq=2, tv=2, output-proj of q-tile 0
      window 2: tq=3, tv=3, output-proj of q-tile 1
      window 3: output-proj of q-tile 2;   tail: output-proj of q-tile 3
    """
    pers_cm = tc.tile_pool(name="pers", bufs=1)
    pers = pers_cm.__enter__()
    KT = pers.tile([128, 4, S], BF16)            # [part=eh%128, et, t]
    V65 = pers.tile([128, NKT, HPC, 65], BF16)   # [t%128, t//128, h, e|1]
    wo_t = pers.tile([128, 4, D], BF16)
    wv_t = pers.tile([128, NDK, EH], BF16)
    bq_t = pers.tile([128, 4], F32)
    bk_t = pers.tile([128, 4], F32)
    bv_t = pers.tile([128, EH], F32)
    ones_t = pers.tile([128, 1], BF16)

    pw_cm = tc.tile_pool(name="pw", bufs=2)
    pw = pw_cm.__enter__()
    px_cm = tc.tile_pool(name="px", bufs=3)
    px = px_cm.__enter__()
    pq_cm = tc.tile_pool(name="pq", bufs=2)
    pq = pq_cm.__enter__()
    pcw_cm = tc.tile_pool(name="pcw", bufs=3)
    pcw = pcw_cm.__enter__()
    ppt_cm = tc.tile_pool(name="ppt", bufs=3)
    ppt = ppt_cm.__enter__()
    pnrm_cm = tc.tile_pool(name="pnrm", bufs=1)
    pnrm = pnrm_cm.__enter__()
    py_cm = tc.tile_pool(name="py", bufs=2)
    py = py_cm.__enter__()
    pp_cm = tc.tile_pool(name="pp", bufs=2, space="PSUM")
    pp = pp_cm.__enter__()
    psc_cm = tc.tile_pool(name="psc", bufs=2, space="PSUM")
    psc = psc_cm.__enter__()
    pav_cm = tc.tile_pool(name="pav", bufs=1, space="PSUM")
    pav = pav_cm.__enter__()

    qwin = {}    # tq -> [128, 4, 512] Q^T window tile
    ctxw = {}    # qt -> [128, 4, 512] ctx^T window tile
    state = {}   # live w/x tiles for the chunk being emitted

    # ---- chunk closures ----
    def qk_chunks(tq):
        # half-tile loads (dk 0-3 / 4-7) so bufs=3 gives cross-chunk prefetch
        def load(w_d, x_v, kind, half):
            def f():
                hs = slice(half * 4, half * 4 + 4)
                w_t = pw.tile([128, 4, EH], BF16, tag="w",
                              name=f"w_{kind}{tq}{half}")
                nc.sync.dma_start(w_t[:], w_d[:, hs, :])
                x_t = px.tile([128, 4, 512], BF16, tag="x",
                              name=f"x_{kind}{tq}{half}")
                nc.sync.dma_start(x_t[:], x_v[:, hs, tq * 512:(tq + 1) * 512])
                state[f"w{half}"], state[f"x{half}"] = w_t, x_t
                if kind == "q" and half == 0:
                    qwin[tq] = pq.tile([128, 4, 512], BF16, tag="qw",
                                       name=f"qw{tq}")
            return f

        def mmgroup(et, kind):
            def f():
                ps_t = pp.tile([128, 512], F32, tag="pj", name=f"ps_{kind}{tq}_{et}")
                for dk in range(NDK):
                    w_t = state[f"w{dk // 4}"]
                    x_t = state[f"x{dk // 4}"]
                    mi = nc.tensor.matmul(ps_t[:],
                                     w_t[:, dk % 4, et * 128:(et + 1) * 128],
                                     x_t[:, dk % 4, :],
                                     start=(dk == 0), stop=(dk == NDK - 1))
                    MM_NAMES["pj"].add(mi.ins.name)
                if kind == "q":
                    nc.vector.tensor_tensor(
                        qwin[tq][:, et, :], ps_t[:],
                        bq_t[:, et:et + 1].to_broadcast([128, 512]), AL.add)
                else:
                    nc.vector.tensor_tensor(
                        KT[:, et, tq * 512:(tq + 1) * 512], ps_t[:],
                        bk_t[:, et:et + 1].to_broadcast([128, 512]), AL.add)
            return f

        out = []
        for kind, w_d, x_v in (("q", wqT_d, xq_v), ("k", wkT_d, xk_v)):
            for half in (0, 1):
                g = load(w_d, x_v, kind, half)
                g.mms = 0
                out.append(g)
            for et in range(4):
                g = mmgroup(et, kind)
                g.mms = 8
                out.append(g)
        return out

    def v_chunks(tv):
        def load(half):
            def f():
                hs = slice(half * 4, half * 4 + 4)
                x_t = px.tile([128, 4, 512], BF16, tag="x", name=f"x_v{tv}{half}")
                nc.sync.dma_start(x_t[:], xv_v[:, hs, tv * 512:(tv + 1) * 512])
                state[f"x{half}"] = x_t
            return f

        def mmgroup(tl):
            def f():
                tt = tv * 4 + tl
                ps_t = pp.tile([128, 512], F32, tag="pj", name=f"ps_v{tt}")
                for dk in range(NDK):
                    x_t = state[f"x{dk // 4}"]
                    mi = nc.tensor.matmul(ps_t[:],
                                     x_t[:, dk % 4, tl * 128:(tl + 1) * 128],
                                     wv_t[:, dk, :],
                                     start=(dk == 0), stop=(dk == NDK - 1))
                    MM_NAMES["pj"].add(mi.ins.name)
                nc.vector.tensor_tensor(
                    V65[:, tt, :, 0:64],
                    ps_t.rearrange("p (h e) -> p h e", h=HPC),
                    bv_t.rearrange("p (h e) -> p h e", h=HPC), AL.add)
            return f

        out = []
        for half in (0, 1):
            g = load(half)
            g.mms = 0
            out.append(g)
        for tl in range(4):
            g = mmgroup(tl)
            g.mms = 8
            out.append(g)
        return out

    def p3_chunks(qt):
        p3state = {}

        def half_a(tl, mc):
            def f():
                cw = ctxw[qt]
                tt = qt * 4 + tl
                ps_t = pp.tile([128, 512], F32, tag="pj", name=f"ps_o{tt}_{mc}")
                p3state[(tl, mc)] = ps_t
                for hp in range(2):
                    mi = nc.tensor.matmul(ps_t[:],
                                          cw[:, hp, tl * 128:(tl + 1) * 128],
                                          wo_t[:, hp, mc * 512:(mc + 1) * 512],
                                          start=(hp == 0), stop=False)
                    MM_NAMES["p3"].add(mi.ins.name)
            f.mms = 2
            return f

        def half_b(tl, mc):
            def f():
                cw = ctxw[qt]
                tt = qt * 4 + tl
                ps_t = p3state.pop((tl, mc))
                for hp in range(2, 4):
                    mi = nc.tensor.matmul(ps_t[:],
                                          cw[:, hp, tl * 128:(tl + 1) * 128],
                                          wo_t[:, hp, mc * 512:(mc + 1) * 512],
                                          start=False, stop=(hp == 3))
                    MM_NAMES["p3"].add(mi.ins.name)
                y_t = py.tile([128, 512], BF16, tag="y", name=f"y{tt}_{mc}")
                nc.vector.tensor_copy(y_t[:], ps_t[:])
                nc.sync.dma_start(
                    y_d[tt * 128:(tt + 1) * 128, mc * 512:(mc + 1) * 512],
                    y_t[:])
            f.mms = 2
            return f

        out = []
        for tl in range(4):
            for mc in range(2):
                out.append(half_a(tl, mc))
                out.append(half_b(tl, mc))
        return out

    # ---- attention window with interleaved work ----
    def window(qt, work):
        klim = k_needed[qt]
        q0 = qt * 512
        cw = pcw.tile([128, 4, 512], BF16, tag="cw", name=f"cw{qt}")
        ctxw[qt] = cw
        qw = qwin[qt]
        n_tiles = 4 * klim
        total_mms = sum(getattr(f, "mms", 4) for f in work) or 1
        wi = 0
        emitted = 0
        done = 0

        av_tiles = {}   # hp -> (av0, av1)

        def emit_scores(hp, kk):
            """scores matmul pair + exp (+ mask) for (hp, kk); returns the
            closure that emits the delayed AV matmuls."""
            straddle = (qt, kk) in mixed
            qoff = max(0, kk * 128 - q0) if (straddle and mode == "affine") else 0
            moff = 128 if qoff >= 128 else 0
            s_t = psc.tile([128, 2, 512], F32, tag="sc")
            for j in range(2):
                mi = nc.tensor.matmul(
                    s_t[:, j, moff:512],
                    KT[j * 64:(j + 1) * 64, hp, kk * 128:(kk + 1) * 128],
                    qw[j * 64:(j + 1) * 64, hp, moff:512],
                    start=True, stop=True, tile_position=(j * 64, 0))
                MM_NAMES[f"sc{j}"].add(mi.ins.name)
            p_t = ppt.tile([128, 2, 512], BF16, tag="pt")
            nc.scalar.activation(p_t[:, :, qoff:512], s_t[:, :, qoff:512],
                                 AF.Exp, scale=float(SCALE))
            if straddle:
                if mode == "affine":
                    nc.gpsimd.affine_select(
                        p_t[:], p_t[:], pattern=[[0, 2], [1, 512]],
                        compare_op=AL.is_ge, fill=0.0,
                        base=q0 - kk * 128, channel_multiplier=-1)
                else:
                    sel_t = ppt.tile([128, 512], BF16, tag="sel")
                    nc.sync.dma_start(sel_t[:], mT_v[:, kk, q0:q0 + 512])
                    nc.vector.tensor_tensor(
                        p_t[:], p_t[:],
                        sel_t[:, None, :].to_broadcast([128, 2, 512]),
                        AL.mult)

            def emit_av():
                if kk == 0:
                    # allocate at first use: allocating earlier would put the
                    # pool release boundary (bufs=1, aliases the previous
                    # hp's accumulators) before the previous hp's final AV
                    # and norm reads - a race.
                    av_tiles[hp] = (
                        pav.tile([65, 512], F32, tag="av0",
                                 name=f"av0_{qt}_{hp}"),
                        pav.tile([65, 512], F32, tag="av1",
                                 name=f"av1_{qt}_{hp}"))
                av0, av1 = av_tiles[hp]
                for j, av in ((0, av0), (1, av1)):
                    mi = nc.tensor.matmul(av[:], V65[:, kk, 2 * hp + j, :],
                                     p_t[:, j, :],
                                     start=(kk == 0), stop=(kk == klim - 1))
                    MM_NAMES["av"].add(mi.ins.name)
            return emit_av

        def emit_norm(hp):
            # copy to SBUF first so the psum accumulators free early;
            # head1 lands at partitions 64-127 to keep TT bases aligned
            av0, av1 = av_tiles.pop(hp)
            avc = pnrm.tile([128, 512], F32, tag="avc", bufs=1)
            lin = pnrm.tile([1, 2, 512], F32, tag="lin")
            nc.vector.tensor_copy(avc[0:64, :], av0[0:64, :])
            nc.vector.tensor_copy(avc[64:128, :], av1[0:64, :])
            nc.vector.tensor_copy(lin[:, 0, :], av0[64:65, :])
            nc.vector.tensor_copy(lin[:, 1, :], av1[64:65, :])
            lrec = pnrm.tile([1, 2, 512], F32, tag="lrec")
            rec_bc = pnrm.tile([128, 2, 512], F32, tag="rbc")
            # rec_bc[0:1] doubles as reciprocal scratch (pbcast overwrites it)
            nc.vector.reciprocal_approx_accurate(lrec[:], lin[:],
                                                 rec_bc[0:1, :, :])
            nc.gpsimd.partition_broadcast(rec_bc[:], lrec[0:1, :, :])
            nc.vector.tensor_tensor(cw[0:64, hp, :],
                                    avc[0:64, :], rec_bc[0:64, 0, :], AL.mult)
            nc.vector.tensor_tensor(cw[64:128, hp, :],
                                    avc[64:128, :], rec_bc[64:128, 1, :], AL.mult)

        # flat software pipeline over (hp, kk): AV for slot i is emitted
        # after the scores for slot i+1 plus an interleave quantum, so the
        # exp/mask latency hides behind queued PE work.
        slots = [(hp, kk) for hp in range(4) for kk in range(klim)]
        pending_av = None
        pending_hp = None
        for hp, kk in slots:
            av_f = emit_scores(hp, kk)
            done += 1
            target = done * total_mms / n_tiles
            while wi < len(work) and emitted < target:
                emitted += getattr(work[wi], "mms", 4)
                work[wi]()
                wi += 1
            if pending_av is not None:
                pending_av()
                if pending_hp is not None:
                    emit_norm(pending_hp)
                    pending_hp = None
            pending_av = av_f
            if kk == klim - 1:
                pending_hp = hp
        pending_av()
        emit_norm(pending_hp)
        while wi < len(work):
            work[wi]()
            wi += 1

    # ---- prologue: bias DMAs first (tiny; the tile framework only sees a
    # write->read dependency if the write is EMITTED before the read), then
    # Q/K for tq=0 (their DMAs gate the first matmul), then the heavy
    # persistent-tile DMAs, then V for tv=0 ----
    nc.sync.dma_start(bq_t[:], bq_d)
    nc.sync.dma_start(bk_t[:], bk_d)
    for f in qk_chunks(0):
        f()
    nc.sync.dma_start(bv_t[:], bv_d)
    nc.sync.dma_start(ones_t[:], ones_d)
    nc.sync.dma_start(wv_t[:], wvT_d)
    nc.vector.tensor_copy(V65[:, :, :, 64:65],
                          ones_t[:, 0:1].to_broadcast([128, NKT, HPC, 1]))
    for f in v_chunks(0):
        f()

    # ---- windows ----
    def wo_load():
        nc.sync.dma_start(wo_t[:], woT_d)
    wo_load.mms = 0

    for qt in range(NQT):
        work = []
        if qt == 0:
            work.append(wo_load)
        if qt + 1 < NQT:
            work += qk_chunks(qt + 1)
            work += v_chunks(qt + 1)
        if qt == 2:
            work += p3_chunks(0)
        elif qt == 3:
            work += p3_chunks(1)
            work += p3_chunks(2)
        window(qt, work)
    for f in p3_chunks(NQT - 1):
        f()

    for cm in (pav_cm, psc_cm, pp_cm, py_cm, pnrm_cm, ppt_cm, pcw_cm, pq_cm,
               px_cm, pw_cm, pers_cm):
        cm.__exit__(None, None, None)


def _analyze_mask(mask):
    """Classify the mask and derive the per-q-tile k-tile structure."""
    m = np.asarray(mask)
    iota = np.arange(S)
    n = m.sum(axis=2)                     # [B, S] count of ones per row
    prefix_ok = bool((m == (iota[None, None, :] < n[..., None])).all())
    causal = prefix_ok and bool((n == iota[None, :] + 1).all())
    allones = bool((m == 1).all())

    k_needed, k_full, mixed = [], [], set()
    if allones:
        mode = "affine"   # no mask ops at all
        k_needed = [NKT] * NQT
        k_full = [NKT] * NQT
    elif causal:
        mode = "affine"
        for qt in range(NQT):
            k_needed.append(4 * qt + 4)
            k_full.append(4 * qt)
            for kk in range(4 * qt, 4 * qt + 4):
                mixed.add((qt, kk))
    else:
        mode = "dense"
        for qt in range(NQT):
            sl = m[:, qt * 512:(qt + 1) * 512, :]       # [B, 512, S]
            need = 0
            full = NKT
            for kk in range(NKT):
                blk = sl[:, :, kk * 128:(kk + 1) * 128]
                if blk.any():
                    need = kk + 1
                if not blk.all():
                    full = min(full, kk)
            need = max(need, 1)
            full = min(full, need)
            k_needed.append(need)
            k_full.append(full)
            for kk in range(full, need):
                blk = sl[:, :, kk * 128:(kk + 1) * 128]
                if not blk.all():
                    mixed.add((qt, kk))
    return mode, tuple(k_needed), tuple(k_full), frozenset(mixed)


def _prep_inputs(x_q, x_k, x_v, mask, wq, wk, wv, bq, bk, bv, wo, mode):
    """Build the 8 per-core input dicts."""
    import ml_dtypes
    f32 = np.float32
    bf16 = ml_dtypes.bfloat16
    in_maps = []
    ones1 = np.ones((128, 1), bf16)
    for core in range(NC):
        b, g = divmod(core, G)
        hs = slice(g * HPC, (g + 1) * HPC)
        im = {
            "xqT": np.ascontiguousarray(np.asarray(x_q[b], f32).T).astype(bf16),
            "xkT": np.ascontiguousarray(np.asarray(x_k[b], f32).T).astype(bf16),
            "xvT": np.ascontiguousarray(np.asarray(x_v[b], f32).T).astype(bf16),
            "ones1": ones1,
        }
        for name, w in (("wqT", wq), ("wkT", wk), ("wvT", wv)):
            # [H, DH, D] group slice -> [D, EH] -> [128, NDK, EH] with d = dk*128+p
            wt = np.asarray(w[hs], f32).transpose(2, 0, 1).reshape(D, EH)
            im[name] = np.ascontiguousarray(wt.reshape(NDK, 128, EH))\
                .transpose(1, 0, 2).astype(bf16)
        for name, bb in (("bq", bq), ("bk", bk)):
            flat = np.asarray(bb[hs], f32).reshape(EH)
            im[name] = np.ascontiguousarray(flat.reshape(4, 128).T)
        im["bv"] = np.broadcast_to(np.asarray(bv[hs], f32).reshape(1, EH),
                                   (128, EH)).copy()
        woT = np.asarray(wo[:, g * EH:(g + 1) * EH], f32).T   # [EH, D]
        im["woT"] = np.ascontiguousarray(woT.reshape(4, 128, D))\
            .transpose(1, 0, 2).astype(bf16)
        if mode == "dense":
            im["maskT"] = np.ascontiguousarray(
                np.asarray(mask[b], f32).T).astype(bf16)
        in_maps.append(im)
    return in_maps


def _run(x_q, x_k, x_v, mask, wq, wk, wv, bq, bk, bv, wo, bo,
         trace=False, trace_cores=None):
    mode, k_needed, k_full, mixed = _analyze_mask(mask)
    key = (mode, k_needed, k_full, mixed)
    if key not in _cache:
        _cache[key] = _build(mode, k_needed, k_full, mixed)
    nc = _cache[key]
    in_maps = _prep_inputs(x_q, x_k, x_v, mask, wq, wk, wv, bq, bk, bv, wo, mode)
    res = run_bass_kernel_spmd(nc, in_maps, core_ids=list(range(NC)),
                               trace=trace, trace_cores=trace_cores)
    bo = np.asarray(bo, np.float32)
    out = np.empty((B, S, D), np.float32)
    for b in range(B):
        out[b] = (res.results[2 * b]["y"].astype(np.float32)
                  + res.results[2 * b + 1]["y"].astype(np.float32) + bo)
    return out, res


def kernel(x_q, x_k, x_v, mask, wq, wk, wv, bq, bk, bv, wo, bo):
    out, _ = _run(x_q, x_k, x_v, mask, wq, wk, wv, bq, bk, bv, wo, bo)
    return out


# revision 16
# speedup vs baseline: 1.1887x; 1.0238x over previous
"""MultiHeadAttention (B=4, S=2048, d_model=1024, H=16, dh=64) on 8 trn2 cores.

Sharding: core (b, g) = batch b in 0..3, head-group g in 0..1 (8 heads each).
Each core computes, for its (b, g):
  Q^T, K^T  [512, 2048] head-dim-major; V [2048, 512] token-major (+ ones col)
  transposed scores S^T = K^T_tile.T @ Q^T per (head, k-tile 128, q-tile 512)
  P = exp(S^T / 8) (no max subtraction; scores are O(1)); causal masking via
  affine_select (skip fully-masked k-tiles entirely)
  fused AV+rowsum: lhsT = [V | 1] -> psum [65, 512]; ctx normalized by 1/l via
  gpsimd partition_broadcast + one tensor_tensor (PSUM operand)
  partial output projection y_partial = ctx^T.T @ wo[:, group].T
Host sums the two groups' partials per batch and adds bo.

All matmul operands are bfloat16 (full-rate PE, half the DMA/LDWEIGHTS
bytes of fp32r; PSUM accumulation stays fp32).

Schedule: engine queues are in-order, so latency is hidden via emission
order: the AV matmuls for k-tile kk are emitted one slot late (after the
scores for kk+1 plus an interleave quantum) so the exp (scalar engine,
~1us) and affine_select (gpsimd) latencies hide behind queued PE work.
Straddle (diagonal) tiles trim the scores matmul to [moff:512] (moff <=
128; larger PSUM write offsets miscompute) and the exp to [qoff:512];
affine_select zero-fills the full tile so the AV reads no garbage.
NOTE: pool tiles must be allocated at first USE, and a tile's DMA write
must be EMITTED before any reader - the tile framework tracks deps and
pool release boundaries by emission order.
"""
import sys
sys.path.insert(0, "/opt/trn_rl_repo")

import numpy as np

import concourse.bass as bass
import concourse.mybir as mybir
import concourse.tile as tile
from concourse import bacc
from concourse.bass_utils import run_bass_kernel_spmd

F32 = mybir.dt.float32
F32R = mybir.dt.float32r
BF16 = mybir.dt.bfloat16
AF = mybir.ActivationFunctionType
AL = mybir.AluOpType

B, S, D, H, DH = 4, 2048, 1024, 16, 64
NC = 8
G = 2              # head groups (cores per batch)
HPC = H // G       # 8 heads per core
EH = HPC * DH      # 512
NQT = S // 512     # 4 q-tiles
NKT = S // 128     # 16 k-tiles
NDK = D // 128     # 8 contraction subtiles
SCALE = 1.0 / np.sqrt(DH)

_cache = {}
MM_NAMES = {"sc0": set(), "sc1": set(), "av": set(), "pj": set(), "p3": set()}


def _build(mode, k_needed, k_full, mixed):
    """Build the per-core Bass program.

    mode: "affine" (causal / prefix masks expressible as q-k>=const... strictly
          the tril case) or "dense" (per-element 0/1 mask multiply from DRAM).
    k_needed[qt]: number of leading k-tiles to compute for q-tile qt.
    k_full[qt]:   k-tiles below this index need no masking.
    mixed: set of (qt, kk) needing a mask op (affine: affine_select;
           dense: sel-tile multiply).
    """
    nc = bacc.Bacc("TRN2", target_bir_lowering=False, debug=False, num_devices=NC)

    xqT_d = nc.dram_tensor("xqT", [D, S], BF16, kind="ExternalInput").ap()
    xkT_d = nc.dram_tensor("xkT", [D, S], BF16, kind="ExternalInput").ap()
    xvT_d = nc.dram_tensor("xvT", [D, S], BF16, kind="ExternalInput").ap()
    wqT_d = nc.dram_tensor("wqT", [128, NDK, EH], BF16, kind="ExternalInput").ap()
    wkT_d = nc.dram_tensor("wkT", [128, NDK, EH], BF16, kind="ExternalInput").ap()
    wvT_d = nc.dram_tensor("wvT", [128, NDK, EH], BF16, kind="ExternalInput").ap()
    bq_d = nc.dram_tensor("bq", [128, 4], F32, kind="ExternalInput").ap()
    bk_d = nc.dram_tensor("bk", [128, 4], F32, kind="ExternalInput").ap()
    bv_d = nc.dram_tensor("bv", [128, EH], F32, kind="ExternalInput").ap()
    woT_d = nc.dram_tensor("woT", [128, 4, D], BF16, kind="ExternalInput").ap()
    ones_d = nc.dram_tensor("ones1", [128, 1], BF16, kind="ExternalInput").ap()
    if mode == "dense":
        mT_d = nc.dram_tensor("maskT", [S, S], BF16, kind="ExternalInput").ap()
        mT_v = mT_d.rearrange("(kt p) q -> p kt q", p=128)
    y_d = nc.dram_tensor("y", [S, D], BF16, kind="ExternalOutput").ap()

    xq_v = xqT_d.rearrange("(dk p) t -> p dk t", p=128)
    xk_v = xkT_d.rearrange("(dk p) t -> p dk t", p=128)
    xv_v = xvT_d.rearrange("(dk p) t -> p dk t", p=128)

    with tile.TileContext(nc) as tc:
        with nc.allow_low_precision(reason="fp32r storage has fp32 width"):
            _body(nc, tc, mode, k_needed, k_full, mixed,
                  xq_v, xk_v, xv_v, wqT_d, wkT_d, wvT_d,
                  bq_d, bk_d, bv_d, woT_d, ones_d,
                  mT_v if mode == "dense" else None, y_d)
    nc.compile()
    return nc


def _body(nc, tc, mode, k_needed, k_full, mixed,
          xq_v, xk_v, xv_v, wqT_d, wkT_d, wvT_d,
          bq_d, bk_d, bv_d, woT_d, ones_d, mT_v, y_d):
    """Interleaved schedule: Q/K/V projection chunks and output-projection
    chunks are emitted *between* attention k-tiles so the PE stays dense
    (HAM warm) and phases overlap.

    Window qt runs attention for all 4 head-pairs on q-tile qt, interleaving:
      window 0: Q/K proj for tq=1, V proj for tv=1
      window 1: tq=2, tv=2, output-proj of q-tile 0
      window 2: tq=3, tv=3, output-proj of q-tile 1
      window 3: output-proj of q-tile 2;   tail: output-proj of q-tile 3
    """
    pers_cm = tc.tile_pool(name="pers", bufs=1)
    pers = pers_cm.__enter__()
    KT = pers.tile([128, 4, S], BF16)            # [part=eh%128, et, t]
    V65 = pers.tile([128, NKT, HPC, 65], BF16)   # [t%128, t//128, h, e|1]
    wo_t = pers.tile([128, 4, D], BF16)
    wv_t = pers.tile([128, NDK, EH], BF16)
    bq_t = pers.tile([128, 4], F32)
    bk_t = pers.tile([128, 4], F32)
    bv_t = pers.tile([128, EH], F32)
    ones_t = pers.tile([128, 1], BF16)

    pw_cm = tc.tile_pool(name="pw", bufs=2)
    pw = pw_cm.__enter__()
    px_cm = tc.tile_pool(name="px", bufs=3)
    px = px_cm.__enter__()
    pq_cm = tc.tile_pool(name="pq", bufs=2)
    pq = pq_cm.__enter__()
    pcw_cm = tc.tile_pool(name="pcw", bufs=3)
    pcw = pcw_cm.__enter__()
    ppt_cm = tc.tile_pool(name="ppt", bufs=3)
    ppt = ppt_cm.__enter__()
    pnrm_cm = tc.tile_pool(name="pnrm", bufs=1)
    pnrm = pnrm_cm.__enter__()
    py_cm = tc.tile_pool(name="py", bufs=2)
    py = py_cm.__enter__()
    pp_cm = tc.tile_pool(name="pp", bufs=2, space="PSUM")
    pp = pp_cm.__enter__()
    psc_cm = tc.tile_pool(name="psc", bufs=2, space="PSUM")
    psc = psc_cm.__enter__()
    pav_cm = tc.tile_pool(name="pav", bufs=1, space="PSUM")
    pav = pav_cm.__enter__()

    qwin = {}    # tq -> [128, 4, 512] Q^T window tile
    ctxw = {}    # qt -> [128, 4, 512] ctx^T window tile
    state = {}   # live w/x tiles for the chunk being emitted

    # ---- chunk closures ----
    def qk_chunks(tq):
        # half-tile loads (dk 0-3 / 4-7) so bufs=3 gives cross-chunk prefetch
        def load(w_d, x_v, kind, half):
            def f():
                hs = slice(half * 4, half * 4 + 4)
                w_t = pw.tile([128, 4, EH], BF16, tag="w",
                              name=f"w_{kind}{tq}{half}")
                nc.sync.dma_start(w_t[:], w_d[:, hs, :])
                x_t = px.tile([128, 4, 512], BF16, tag="x",
                              name=f"x_{kind}{tq}{half}")
                nc.sync.dma_start(x_t[:], x_v[:, hs, tq * 512:(tq + 1) * 512])
                state[f"w{half}"], state[f"x{half}"] = w_t, x_t
                if kind == "q" and half == 0:
                    qwin[tq] = pq.tile([128, 4, 512], BF16, tag="qw",
                                       name=f"qw{tq}")
            return f

        def mmgroup(et, kind):
            def f():
                ps_t = pp.tile([128, 512], F32, tag="pj", name=f"ps_{kind}{tq}_{et}")
                for dk in range(NDK):
                    w_t = state[f"w{dk // 4}"]
                    x_t = state[f"x{dk // 4}"]
                    mi = nc.tensor.matmul(ps_t[:],
                                     w_t[:, dk % 4, et * 128:(et + 1) * 128],
                                     x_t[:, dk % 4, :],
                                     start=(dk == 0), stop=(dk == NDK - 1))
                    MM_NAMES["pj"].add(mi.ins.name)
                if kind == "q":
                    nc.vector.tensor_tensor(
                        qwin[tq][:, et, :], ps_t[:],
                        bq_t[:, et:et + 1].to_broadcast([128, 512]), AL.add)
                else:
                    nc.vector.tensor_tensor(
                        KT[:, et, tq * 512:(tq + 1) * 512], ps_t[:],
                        bk_t[:, et:et + 1].to_broadcast([128, 512]), AL.add)
            return f

        out = []
        for kind, w_d, x_v in (("q", wqT_d, xq_v), ("k", wkT_d, xk_v)):
            for half in (0, 1):
                g = load(w_d, x_v, kind, half)
                g.mms = 0
                out.append(g)
            for et in range(4):
                g = mmgroup(et, kind)
                g.mms = 8
                out.append(g)
        return out

    def v_chunks(tv):
        def load(half):
            def f():
                hs = slice(half * 4, half * 4 + 4)
                x_t = px.tile([128, 4, 512], BF16, tag="x", name=f"x_v{tv}{half}")
                nc.sync.dma_start(x_t[:], xv_v[:, hs, tv * 512:(tv + 1) * 512])
                state[f"x{half}"] = x_t
            return f

        def mmgroup(tl):
            def f():
                tt = tv * 4 + tl
                ps_t = pp.tile([128, 512], F32, tag="pj", name=f"ps_v{tt}")
                for dk in range(NDK):
                    x_t = state[f"x{dk // 4}"]
                    mi = nc.tensor.matmul(ps_t[:],
                                     x_t[:, dk % 4, tl * 128:(tl + 1) * 128],
                                     wv_t[:, dk, :],
                                     start=(dk == 0), stop=(dk == NDK - 1))
                    MM_NAMES["pj"].add(mi.ins.name)
                nc.vector.tensor_tensor(
                    V65[:, tt, :, 0:64],
                    ps_t.rearrange("p (h e) -> p h e", h=HPC),
                    bv_t.rearrange("p (h e) -> p h e", h=HPC), AL.add)
            return f

        out = []
        for half in (0, 1):
            g = load(half)
            g.mms = 0
            out.append(g)
        for tl in range(4):
            g = mmgroup(tl)
            g.mms = 8
            out.append(g)
        return out

    def p3_chunks(qt):
        p3state = {}

        def half_a(tl, mc):
            def f():
                cw = ctxw[qt]
                tt = qt * 4 + tl
                ps_t = pp.tile([128, 512], F32, tag="pj", name=f"ps_o{tt}_{mc}")
                p3state[(tl, mc)] = ps_t
                for hp in range(2):
                    mi = nc.tensor.matmul(ps_t[:],
                                          cw[:, hp, tl * 128:(tl + 1) * 128],
                                          wo_t[:, hp, mc * 512:(mc + 1) * 512],
                                          start=(hp == 0), stop=False)
                    MM_NAMES["p3"].add(mi.ins.name)
            f.mms = 2
            return f

        def half_b(tl, mc):
            def f():
                cw = ctxw[qt]
                tt = qt * 4 + tl
                ps_t = p3state.pop((tl, mc))
                for hp in range(2, 4):
                    mi = nc.tensor.matmul(ps_t[:],
                                          cw[:, hp, tl * 128:(tl + 1) * 128],
                                          wo_t[:, hp, mc * 512:(mc + 1) * 512],
                                          start=False, stop=(hp == 3))
                    MM_NAMES["p3"].add(mi.ins.name)
                y_t = py.tile([128, 512], BF16, tag="y", name=f"y{tt}_{mc}")
                nc.vector.tensor_copy(y_t[:], ps_t[:])
                nc.sync.dma_start(
                    y_d[tt * 128:(tt + 1) * 128, mc * 512:(mc + 1) * 512],
                    y_t[:])
            f.mms = 2
            return f

        out = []
        for tl in range(4):
            for mc in range(2):
                out.append(half_a(tl, mc))
                out.append(half_b(tl, mc))
        return out

    # ---- attention window with interleaved work ----
    def window(qt, work):
        klim = k_needed[qt]
        q0 = qt * 512
        cw = pcw.tile([128, 4, 512], BF16, tag="cw", name=f"cw{qt}")
        ctxw[qt] = cw
        qw = qwin[qt]
        n_tiles = 4 * klim
        total_mms = sum(getattr(f, "mms", 4) for f in work) or 1
        wi = 0
        emitted = 0
        done = 0

        av_tiles = {}   # hp -> (av0, av1)

        def emit_scores(hp, kk):
            """scores matmul pair + exp (+ mask) for (hp, kk); returns the
            closure that emits the delayed AV matmuls."""
            straddle = (qt, kk) in mixed
            qoff = max(0, kk * 128 - q0) if (straddle and mode == "affine") else 0
            moff = 128 if qoff >= 128 else 0
            s_t = psc.tile([128, 2, 512], F32, tag="sc")
            for j in range(2):
                mi = nc.tensor.matmul(
                    s_t[:, j, moff:512],
                    KT[j * 64:(j + 1) * 64, hp, kk * 128:(kk + 1) * 128],
                    qw[j * 64:(j + 1) * 64, hp, moff:512],
                    start=True, stop=True, tile_position=(j * 64, 0))
                MM_NAMES[f"sc{j}"].add(mi.ins.name)
            p_t = ppt.tile([128, 2, 512], BF16, tag="pt")
            nc.scalar.activation(p_t[:, :, qoff:512], s_t[:, :, qoff:512],
                                 AF.Exp, scale=float(SCALE))
            if straddle:
                if mode == "affine":
                    nc.gpsimd.affine_select(
                        p_t[:], p_t[:], pattern=[[0, 2], [1, 512]],
                        compare_op=AL.is_ge, fill=0.0,
                        base=q0 - kk * 128, channel_multiplier=-1)
                else:
                    sel_t = ppt.tile([128, 512], BF16, tag="sel")
                    nc.sync.dma_start(sel_t[:], mT_v[:, kk, q0:q0 + 512])
                    nc.vector.tensor_tensor(
                        p_t[:], p_t[:],
                        sel_t[:, None, :].to_broadcast([128, 2, 512]),
                        AL.mult)

            def emit_av():
                if kk == 0:
                    # allocate at first use: allocating earlier would put the
                    # pool release boundary (bufs=1, aliases the previous
                    # hp's accumulators) before the previous hp's final AV
                    # and norm reads - a race.
                    av_tiles[hp] = (
                        pav.tile([65, 512], F32, tag="av0",
                                 name=f"av0_{qt}_{hp}"),
                        pav.tile([65, 512], F32, tag="av1",
                                 name=f"av1_{qt}_{hp}"))
                av0, av1 = av_tiles[hp]
                for j, av in ((0, av0), (1, av1)):
                    mi = nc.tensor.matmul(av[:], V65[:, kk, 2 * hp + j, :],
                                     p_t[:, j, :],
                                     start=(kk == 0), stop=(kk == klim - 1))
                    MM_NAMES["av"].add(mi.ins.name)
            return emit_av

        def emit_norm(hp):
            # copy to SBUF first so the psum accumulators free early;
            # head1 lands at partitions 64-127 to keep TT bases aligned
            av0, av1 = av_tiles.pop(hp)
            avc = pnrm.tile([128, 512], F32, tag="avc", bufs=1)
            lin = pnrm.tile([1, 2, 512], F32, tag="lin")
            nc.vector.tensor_copy(avc[0:64, :], av0[0:64, :])
            nc.vector.tensor_copy(avc[64:128, :], av1[0:64, :])
            nc.vector.tensor_copy(lin[:, 0, :], av0[64:65, :])
            nc.vector.tensor_copy(lin[:, 1, :], av1[64:65, :])
            lrec = pnrm.tile([1, 2, 512], F32, tag="lrec")
            rec_bc = pnrm.tile([128, 2, 512], F32, tag="rbc")
            # rec_bc[0:1] doubles as reciprocal scratch (pbcast overwrites it)
            nc.vector.reciprocal_approx_accurate(lrec[:], lin[:],
                                                 rec_bc[0:1, :, :])
            nc.gpsimd.partition_broadcast(rec_bc[:], lrec[0:1, :, :])
            nc.vector.tensor_tensor(cw[0:64, hp, :],
                                    avc[0:64, :], rec_bc[0:64, 0, :], AL.mult)
            nc.vector.tensor_tensor(cw[64:128, hp, :],
                                    avc[64:128, :], rec_bc[64:128, 1, :], AL.mult)

        # flat software pipeline over (hp, kk): AV for slot i is emitted
        # after the scores for slot i+1 plus an interleave quantum, so the
        # exp/mask latency hides behind queued PE work.
        slots = [(hp, kk) for hp in range(4) for kk in range(klim)]
        pending_av = None
        pending_hp = None
        for hp, kk in slots:
            av_f = emit_scores(hp, kk)
            done += 1
            target = done * total_mms / n_tiles
            while wi < len(work) and emitted < target:
                emitted += getattr(work[wi], "mms", 4)
                work[wi]()
                wi += 1
            if pending_av is not None:
                pending_av()
                if pending_hp is not None:
                    emit_norm(pending_hp)
                    pending_hp = None
            pending_av = av_f
            if kk == klim - 1:
                pending_hp = hp
        pending_av()
        emit_norm(pending_hp)
        while wi < len(work):
            work[wi]()
            wi += 1

    # ---- prologue: bias DMAs first (tiny; the tile framework only sees a
    # write->read dependency if the write is EMITTED before the read), then
    # Q/K for tq=0 (their DMAs gate the first matmul), then the heavy
    # persistent-tile DMAs, then V for tv=0 ----
    nc.sync.dma_start(bq_t[:], bq_d)
    nc.sync.dma_start(bk_t[:], bk_d)
    for f in qk_chunks(0):
        f()
    nc.sync.dma_start(bv_t[:], bv_d)
    nc.sync.dma_start(ones_t[:], ones_d)
    nc.sync.dma_start(wv_t[:], wvT_d)
    nc.vector.tensor_copy(V65[:, :, :, 64:65],
                          ones_t[:, 0:1].to_broadcast([128, NKT, HPC, 1]))
    for f in v_chunks(0):
        f()

    # ---- windows ----
    def wo_load():
        nc.sync.dma_start(wo_t[:], woT_d)
    wo_load.mms = 0

    for qt in range(NQT):
        work = []
        if qt == 0:
            work.append(wo_load)
        if qt + 1 < NQT:
            work += qk_chunks(qt + 1)
            work += v_chunks(qt + 1)
        if qt == 2:
            work += p3_chunks(0)
        elif qt == 3:
            work += p3_chunks(1)
            work += p3_chunks(2)
        window(qt, work)
    for f in p3_chunks(NQT - 1):
        f()

    for cm in (pav_cm, psc_cm, pp_cm, py_cm, pnrm_cm, ppt_cm, pcw_cm, pq_cm,
               px_cm, pw_cm, pers_cm):
        cm.__exit__(None, None, None)


def _analyze_mask(mask):
    """Classify the mask and derive the per-q-tile k-tile structure."""
    m = np.asarray(mask)
    iota = np.arange(S)
    n = m.sum(axis=2)                     # [B, S] count of ones per row
    prefix_ok = bool((m == (iota[None, None, :] < n[..., None])).all())
    causal = prefix_ok and bool((n == iota[None, :] + 1).all())
    allones = bool((m == 1).all())

    k_needed, k_full, mixed = [], [], set()
    if allones:
        mode = "affine"   # no mask ops at all
        k_needed = [NKT] * NQT
        k_full = [NKT] * NQT
    elif causal:
        mode = "affine"
        for qt in range(NQT):
            k_needed.append(4 * qt + 4)
            k_full.append(4 * qt)
            for kk in range(4 * qt, 4 * qt + 4):
                mixed.add((qt, kk))
    else:
        mode = "dense"
        for qt in range(NQT):
            sl = m[:, qt * 512:(qt + 1) * 512, :]       # [B, 512, S]
            need = 0
            full = NKT
            for kk in range(NKT):
                blk = sl[:, :, kk * 128:(kk + 1) * 128]
                if blk.any():
                    need = kk + 1
                if not blk.all():
                    full = min(full, kk)
            need = max(need, 1)
            full = min(full, need)
            k_needed.append(need)
            k_full.append(full)
            for kk in range(full, need):
                blk = sl[:, :, kk * 128:(kk + 1) * 128]
                if not blk.all():
                    mixed.add((qt, kk))
    return mode, tuple(k_needed), tuple(k_full), frozenset(mixed)


def _prep_inputs(x_q, x_k, x_v, mask, wq, wk, wv, bq, bk, bv, wo, mode):
    """Build the 8 per-core input dicts."""
    import ml_dtypes
    f32 = np.float32
    bf16 = ml_dtypes.bfloat16
    in_maps = []
    ones1 = np.ones((128, 1), bf16)
    for core in range(NC):
        b, g = divmod(core, G)
        hs = slice(g * HPC, (g + 1) * HPC)
        im = {
            "xqT": np.ascontiguousarray(np.asarray(x_q[b], f32).T).astype(bf16),
            "xkT": np.ascontiguousarray(np.asarray(x_k[b], f32).T).astype(bf16),
            "xvT": np.ascontiguousarray(np.asarray(x_v[b], f32).T).astype(bf16),
            "ones1": ones1,
        }
        for name, w in (("wqT", wq), ("wkT", wk), ("wvT", wv)):
            # [H, DH, D] group slice -> [D, EH] -> [128, NDK, EH] with d = dk*128+p
            wt = np.asarray(w[hs], f32).transpose(2, 0, 1).reshape(D, EH)
            im[name] = np.ascontiguousarray(wt.reshape(NDK, 128, EH))\
                .transpose(1, 0, 2).astype(bf16)
        for name, bb in (("bq", bq), ("bk", bk)):
            flat = np.asarray(bb[hs], f32).reshape(EH)
            im[name] = np.ascontiguousarray(flat.reshape(4, 128).T)
        im["bv"] = np.broadcast_to(np.asarray(bv[hs], f32).reshape(1, EH),
                                   (128, EH)).copy()
        woT = np.asarray(wo[:, g * EH:(g + 1) * EH], f32).T   # [EH, D]
        im["woT"] = np.ascontiguousarray(woT.reshape(4, 128, D))\
            .transpose(1, 0, 2).astype(bf16)
        if mode == "dense":
            im["maskT"] = np.ascontiguousarray(
                np.asarray(mask[b], f32).T).astype(bf16)
        in_maps.append(im)
    return in_maps


def _run(x_q, x_k, x_v, mask, wq, wk, wv, bq, bk, bv, wo, bo,
         trace=False, trace_cores=None):
    mode, k_needed, k_full, mixed = _analyze_mask(mask)
    key = (mode, k_needed, k_full, mixed)
    if key not in _cache:
        _cache[key] = _build(mode, k_needed, k_full, mixed)
    nc = _cache[key]
    in_maps = _prep_inputs(x_q, x_k, x_v, mask, wq, wk, wv, bq, bk, bv, wo, mode)
    res = run_bass_kernel_spmd(nc, in_maps, core_ids=list(range(NC)),
                               trace=trace, trace_cores=trace_cores)
    bo = np.asarray(bo, np.float32)
    out = np.empty((B, S, D), np.float32)
    for b in range(B):
        out[b] = (res.results[2 * b]["y"].astype(np.float32)
                  + res.results[2 * b + 1]["y"].astype(np.float32) + bo)
    return out, res


def kernel(x_q, x_k, x_v, mask, wq, wk, wv, bq, bk, bv, wo, bo):
    out, _ = _run(x_q, x_k, x_v, mask, wq, wk, wv, bq, bk, bv, wo, bo)
    return out


# revision 17
# speedup vs baseline: 1.2272x; 1.0324x over previous
"""MultiHeadAttention (B=4, S=2048, d_model=1024, H=16, dh=64) on 8 trn2 cores.

Sharding: core (b, g) = batch b in 0..3, head-group g in 0..1 (8 heads each).
Each core computes, for its (b, g):
  Q^T, K^T  [512, 2048] head-dim-major; V [2048, 512] token-major (+ ones col)
  transposed scores S^T = K^T_tile.T @ Q^T per (head, k-tile 128, q-tile 512)
  P = exp(S^T / 8) (no max subtraction; scores are O(1)); causal masking via
  affine_select (skip fully-masked k-tiles entirely)
  fused AV+rowsum: lhsT = [V | 1] -> psum [65, 512]; ctx normalized by 1/l via
  gpsimd partition_broadcast + one tensor_tensor (PSUM operand)
  partial output projection y_partial = ctx^T.T @ wo[:, group].T
Host sums the two groups' partials per batch and adds bo.

All matmul operands are bfloat16 (full-rate PE, half the DMA/LDWEIGHTS
bytes of fp32r; PSUM accumulation stays fp32).

Schedule: engine queues are in-order, so latency is hidden via emission
order: the AV matmuls for k-tile kk are emitted one slot late (after the
scores for kk+1 plus an interleave quantum) so the exp (scalar engine,
~1us) and affine_select (gpsimd) latencies hide behind queued PE work.
Straddle (diagonal) tiles trim the scores matmul to [moff:512] (moff <=
128; larger PSUM write offsets miscompute) and the exp to [qoff:512];
affine_select zero-fills the full tile so the AV reads no garbage.
NOTE: pool tiles must be allocated at first USE, and a tile's DMA write
must be EMITTED before any reader - the tile framework tracks deps and
pool release boundaries by emission order.
"""
import sys
sys.path.insert(0, "/opt/trn_rl_repo")

import numpy as np

import concourse.bass as bass
import concourse.mybir as mybir
import concourse.tile as tile
from concourse import bacc
from concourse.bass_utils import run_bass_kernel_spmd

F32 = mybir.dt.float32
F32R = mybir.dt.float32r
BF16 = mybir.dt.bfloat16
AF = mybir.ActivationFunctionType
AL = mybir.AluOpType

B, S, D, H, DH = 4, 2048, 1024, 16, 64
NC = 8
G = 2              # head groups (cores per batch)
HPC = H // G       # 8 heads per core
EH = HPC * DH      # 512
NQT = S // 512     # 4 q-tiles
NKT = S // 128     # 16 k-tiles
NDK = D // 128     # 8 contraction subtiles
SCALE = 1.0 / np.sqrt(DH)

_cache = {}
MM_NAMES = {"sc0": set(), "sc1": set(), "av": set(), "pj": set(), "p3": set()}


def _build(mode, k_needed, k_full, mixed):
    """Build the per-core Bass program.

    mode: "affine" (causal / prefix masks expressible as q-k>=const... strictly
          the tril case) or "dense" (per-element 0/1 mask multiply from DRAM).
    k_needed[qt]: number of leading k-tiles to compute for q-tile qt.
    k_full[qt]:   k-tiles below this index need no masking.
    mixed: set of (qt, kk) needing a mask op (affine: affine_select;
           dense: sel-tile multiply).
    """
    nc = bacc.Bacc("TRN2", target_bir_lowering=False, debug=False, num_devices=NC)

    xqT_d = nc.dram_tensor("xqT", [D, S], BF16, kind="ExternalInput").ap()
    xkT_d = nc.dram_tensor("xkT", [D, S], BF16, kind="ExternalInput").ap()
    xvT_d = nc.dram_tensor("xvT", [D, S], BF16, kind="ExternalInput").ap()
    wqT_d = nc.dram_tensor("wqT", [128, NDK, EH], BF16, kind="ExternalInput").ap()
    wkT_d = nc.dram_tensor("wkT", [128, NDK, EH], BF16, kind="ExternalInput").ap()
    wvT_d = nc.dram_tensor("wvT", [128, NDK, EH], BF16, kind="ExternalInput").ap()
    bq_d = nc.dram_tensor("bq", [128, 4], F32, kind="ExternalInput").ap()
    bk_d = nc.dram_tensor("bk", [128, 4], F32, kind="ExternalInput").ap()
    bv_d = nc.dram_tensor("bv", [128, EH], F32, kind="ExternalInput").ap()
    woT_d = nc.dram_tensor("woT", [128, 4, D], BF16, kind="ExternalInput").ap()
    ones_d = nc.dram_tensor("ones1", [128, 1], BF16, kind="ExternalInput").ap()
    sel4_d = nc.dram_tensor("sel4", [128, 4, 512], BF16, kind="ExternalInput").ap()
    if mode == "dense":
        mT_d = nc.dram_tensor("maskT", [S, S], BF16, kind="ExternalInput").ap()
        mT_v = mT_d.rearrange("(kt p) q -> p kt q", p=128)
    y_d = nc.dram_tensor("y", [S, D], BF16, kind="ExternalOutput").ap()

    xq_v = xqT_d.rearrange("(dk p) t -> p dk t", p=128)
    xk_v = xkT_d.rearrange("(dk p) t -> p dk t", p=128)
    xv_v = xvT_d.rearrange("(dk p) t -> p dk t", p=128)

    with tile.TileContext(nc) as tc:
        with nc.allow_low_precision(reason="fp32r storage has fp32 width"):
            _body(nc, tc, mode, k_needed, k_full, mixed,
                  xq_v, xk_v, xv_v, wqT_d, wkT_d, wvT_d,
                  bq_d, bk_d, bv_d, woT_d, ones_d, sel4_d,
                  mT_v if mode == "dense" else None, y_d)
    nc.compile()
    return nc


def _body(nc, tc, mode, k_needed, k_full, mixed,
          xq_v, xk_v, xv_v, wqT_d, wkT_d, wvT_d,
          bq_d, bk_d, bv_d, woT_d, ones_d, sel4_d, mT_v, y_d):
    """Interleaved schedule: Q/K/V projection chunks and output-projection
    chunks are emitted *between* attention k-tiles so the PE stays dense
    (HAM warm) and phases overlap.

    Window qt runs attention for all 4 head-pairs on q-tile qt, interleaving:
      window 0: Q/K proj for tq=1, V proj for tv=1
      window 1: tq=2, tv=2, output-proj of q-tile 0
      window 2: tq=3, tv=3, output-proj of q-tile 1
      window 3: output-proj of q-tile 2;   tail: output-proj of q-tile 3
    """
    pers_cm = tc.tile_pool(name="pers", bufs=1)
    pers = pers_cm.__enter__()
    KT = pers.tile([128, 4, S], BF16)            # [part=eh%128, et, t]
    V65 = pers.tile([128, NKT, HPC, 65], BF16)   # [t%128, t//128, h, e|1]
    wo_t = pers.tile([128, 4, D], BF16)
    wv_t = pers.tile([128, NDK, EH], BF16)
    bq_t = pers.tile([128, 4], F32)
    bk_t = pers.tile([128, 4], F32)
    bv_t = pers.tile([128, EH], F32)
    ones_t = pers.tile([128, 1], BF16)
    sel4_t = pers.tile([128, 4, 512], BF16)

    pw_cm = tc.tile_pool(name="pw", bufs=2)
    pw = pw_cm.__enter__()
    px_cm = tc.tile_pool(name="px", bufs=3)
    px = px_cm.__enter__()
    pq_cm = tc.tile_pool(name="pq", bufs=2)
    pq = pq_cm.__enter__()
    pcw_cm = tc.tile_pool(name="pcw", bufs=3)
    pcw = pcw_cm.__enter__()
    ppt_cm = tc.tile_pool(name="ppt", bufs=3)
    ppt = ppt_cm.__enter__()
    pnrm_cm = tc.tile_pool(name="pnrm", bufs=1)
    pnrm = pnrm_cm.__enter__()
    py_cm = tc.tile_pool(name="py", bufs=2)
    py = py_cm.__enter__()
    pp_cm = tc.tile_pool(name="pp", bufs=2, space="PSUM")
    pp = pp_cm.__enter__()
    psc_cm = tc.tile_pool(name="psc", bufs=2, space="PSUM")
    psc = psc_cm.__enter__()
    pav_cm = tc.tile_pool(name="pav", bufs=1, space="PSUM")
    pav = pav_cm.__enter__()

    qwin = {}    # tq -> [128, 4, 512] Q^T window tile
    ctxw = {}    # qt -> [128, 4, 512] ctx^T window tile
    state = {}   # live w/x tiles for the chunk being emitted

    # ---- chunk closures ----
    def qk_chunks(tq):
        # half-tile loads (dk 0-3 / 4-7) so bufs=3 gives cross-chunk prefetch
        def load(w_d, x_v, kind, half):
            def f():
                hs = slice(half * 4, half * 4 + 4)
                w_t = pw.tile([128, 4, EH], BF16, tag="w",
                              name=f"w_{kind}{tq}{half}")
                nc.sync.dma_start(w_t[:], w_d[:, hs, :])
                x_t = px.tile([128, 4, 512], BF16, tag="x",
                              name=f"x_{kind}{tq}{half}")
                nc.sync.dma_start(x_t[:], x_v[:, hs, tq * 512:(tq + 1) * 512])
                state[f"w{half}"], state[f"x{half}"] = w_t, x_t
                if kind == "q" and half == 0:
                    qwin[tq] = pq.tile([128, 4, 512], BF16, tag="qw",
                                       name=f"qw{tq}")
            return f

        def mmgroup(et, kind):
            def f():
                ps_t = pp.tile([128, 512], F32, tag="pj", name=f"ps_{kind}{tq}_{et}")
                for dk in range(NDK):
                    w_t = state[f"w{dk // 4}"]
                    x_t = state[f"x{dk // 4}"]
                    mi = nc.tensor.matmul(ps_t[:],
                                     w_t[:, dk % 4, et * 128:(et + 1) * 128],
                                     x_t[:, dk % 4, :],
                                     start=(dk == 0), stop=(dk == NDK - 1))
                    MM_NAMES["pj"].add(mi.ins.name)
                if kind == "q":
                    nc.vector.tensor_tensor(
                        qwin[tq][:, et, :], ps_t[:],
                        bq_t[:, et:et + 1].to_broadcast([128, 512]), AL.add)
                else:
                    nc.vector.tensor_tensor(
                        KT[:, et, tq * 512:(tq + 1) * 512], ps_t[:],
                        bk_t[:, et:et + 1].to_broadcast([128, 512]), AL.add)
            return f

        out = []
        for kind, w_d, x_v in (("q", wqT_d, xq_v), ("k", wkT_d, xk_v)):
            for half in (0, 1):
                g = load(w_d, x_v, kind, half)
                g.mms = 0
                out.append(g)
            for et in range(4):
                g = mmgroup(et, kind)
                g.mms = 8
                out.append(g)
        return out

    def v_chunks(tv):
        def load(half):
            def f():
                hs = slice(half * 4, half * 4 + 4)
                x_t = px.tile([128, 4, 512], BF16, tag="x", name=f"x_v{tv}{half}")
                nc.sync.dma_start(x_t[:], xv_v[:, hs, tv * 512:(tv + 1) * 512])
                state[f"x{half}"] = x_t
            return f

        def mmgroup(tl):
            def f():
                tt = tv * 4 + tl
                ps_t = pp.tile([128, 512], F32, tag="pj", name=f"ps_v{tt}")
                for dk in range(NDK):
                    x_t = state[f"x{dk // 4}"]
                    mi = nc.tensor.matmul(ps_t[:],
                                     x_t[:, dk % 4, tl * 128:(tl + 1) * 128],
                                     wv_t[:, dk, :],
                                     start=(dk == 0), stop=(dk == NDK - 1))
                    MM_NAMES["pj"].add(mi.ins.name)
                nc.vector.tensor_tensor(
                    V65[:, tt, :, 0:64],
                    ps_t.rearrange("p (h e) -> p h e", h=HPC),
                    bv_t.rearrange("p (h e) -> p h e", h=HPC), AL.add)
            return f

        out = []
        for half in (0, 1):
            g = load(half)
            g.mms = 0
            out.append(g)
        for tl in range(4):
            g = mmgroup(tl)
            g.mms = 8
            out.append(g)
        return out

    def p3_chunks(qt):
        p3state = {}

        def half_a(tl, mc):
            def f():
                cw = ctxw[qt]
                tt = qt * 4 + tl
                ps_t = pp.tile([128, 512], F32, tag="pj", name=f"ps_o{tt}_{mc}")
                p3state[(tl, mc)] = ps_t
                for hp in range(2):
                    mi = nc.tensor.matmul(ps_t[:],
                                          cw[:, hp, tl * 128:(tl + 1) * 128],
                                          wo_t[:, hp, mc * 512:(mc + 1) * 512],
                                          start=(hp == 0), stop=False)
                    MM_NAMES["p3"].add(mi.ins.name)
            f.mms = 2
            return f

        def half_b(tl, mc):
            def f():
                cw = ctxw[qt]
                tt = qt * 4 + tl
                ps_t = p3state.pop((tl, mc))
                for hp in range(2, 4):
                    mi = nc.tensor.matmul(ps_t[:],
                                          cw[:, hp, tl * 128:(tl + 1) * 128],
                                          wo_t[:, hp, mc * 512:(mc + 1) * 512],
                                          start=False, stop=(hp == 3))
                    MM_NAMES["p3"].add(mi.ins.name)
                y_t = py.tile([128, 512], BF16, tag="y", name=f"y{tt}_{mc}")
                nc.vector.tensor_copy(y_t[:], ps_t[:])
                nc.sync.dma_start(
                    y_d[tt * 128:(tt + 1) * 128, mc * 512:(mc + 1) * 512],
                    y_t[:])
            f.mms = 2
            return f

        out = []
        for tl in range(4):
            for mc in range(2):
                out.append(half_a(tl, mc))
                out.append(half_b(tl, mc))
        return out

    # ---- attention window with interleaved work ----
    def window(qt, work):
        klim = k_needed[qt]
        q0 = qt * 512
        cw = pcw.tile([128, 4, 512], BF16, tag="cw", name=f"cw{qt}")
        ctxw[qt] = cw
        qw = qwin[qt]
        n_tiles = 4 * klim
        total_mms = sum(getattr(f, "mms", 4) for f in work) or 1
        wi = 0
        emitted = 0
        done = 0

        av_tiles = {}   # hp -> (av0, av1)

        def emit_scores(hp, kk):
            """scores matmul pair + exp (+ mask) for (hp, kk); returns the
            closure that emits the delayed AV matmuls."""
            straddle = (qt, kk) in mixed
            qoff = max(0, kk * 128 - q0) if (straddle and mode == "affine") else 0
            # The first partial use of each of the 3 rotating p_t buffers
            # (window 0, slots 1 and 2) writes the FULL tile: every later
            # slot's stale [0:qoff) region is then a finite old exp value,
            # so the mask multiply below cannot hit NaN garbage.  Never exp
            # an s_t region the matmul did not write (stale PSUM can be NaN).
            force_full = qt == 0 and hp == 0 and kk in (1, 2)
            eoff = 0 if force_full else qoff
            moff = 0 if force_full else (128 if qoff >= 128 else 0)
            s_t = psc.tile([128, 2, 512], F32, tag="sc")
            for j in range(2):
                mi = nc.tensor.matmul(
                    s_t[:, j, moff:512],
                    KT[j * 64:(j + 1) * 64, hp, kk * 128:(kk + 1) * 128],
                    qw[j * 64:(j + 1) * 64, hp, moff:512],
                    start=True, stop=True, tile_position=(j * 64, 0))
                MM_NAMES[f"sc{j}"].add(mi.ins.name)
            p_t = ppt.tile([128, 2, 512], BF16, tag="pt")
            nc.scalar.activation(p_t[:, :, eoff:512], s_t[:, :, eoff:512],
                                 AF.Exp, scale=float(SCALE))
            if straddle:
                if mode == "affine":
                    # causal mask on DVE (bf16 2x mode) with a precomputed
                    # diagonal tile; zeroes the stale [0:qoff) region too.
                    # Keeps the gpsimd queue free for partition_broadcast.
                    qi = qoff // 128
                    nc.vector.tensor_tensor(
                        p_t[:], p_t[:],
                        sel4_t[:, qi:qi + 1, :].to_broadcast([128, 2, 512]),
                        AL.mult)
                else:
                    sel_t = ppt.tile([128, 512], BF16, tag="sel")
                    nc.sync.dma_start(sel_t[:], mT_v[:, kk, q0:q0 + 512])
                    nc.vector.tensor_tensor(
                        p_t[:], p_t[:],
                        sel_t[:, None, :].to_broadcast([128, 2, 512]),
                        AL.mult)

            def emit_av():
                if kk == 0:
                    # allocate at first use: allocating earlier would put the
                    # pool release boundary (bufs=1, aliases the previous
                    # hp's accumulators) before the previous hp's final AV
                    # and norm reads - a race.
                    av_tiles[hp] = (
                        pav.tile([65, 512], F32, tag="av0",
                                 name=f"av0_{qt}_{hp}"),
                        pav.tile([65, 512], F32, tag="av1",
                                 name=f"av1_{qt}_{hp}"))
                av0, av1 = av_tiles[hp]
                for j, av in ((0, av0), (1, av1)):
                    mi = nc.tensor.matmul(av[:], V65[:, kk, 2 * hp + j, :],
                                     p_t[:, j, :],
                                     start=(kk == 0), stop=(kk == klim - 1))
                    MM_NAMES["av"].add(mi.ins.name)
            return emit_av

        def emit_norm(hp):
            # copy to SBUF first so the psum accumulators free early;
            # head1 lands at partitions 64-127 to keep TT bases aligned
            av0, av1 = av_tiles.pop(hp)
            avc = pnrm.tile([128, 512], F32, tag="avc", bufs=1)
            lin = pnrm.tile([1, 2, 512], F32, tag="lin")
            nc.vector.tensor_copy(avc[0:64, :], av0[0:64, :])
            nc.vector.tensor_copy(avc[64:128, :], av1[0:64, :])
            nc.vector.tensor_copy(lin[:, 0, :], av0[64:65, :])
            nc.vector.tensor_copy(lin[:, 1, :], av1[64:65, :])
            lrec = pnrm.tile([1, 2, 512], F32, tag="lrec")
            rec_bc = pnrm.tile([128, 2, 512], F32, tag="rbc")
            # rec_bc[0:1] doubles as reciprocal scratch (pbcast overwrites it)
            nc.vector.reciprocal_approx_accurate(lrec[:], lin[:],
                                                 rec_bc[0:1, :, :])
            nc.gpsimd.partition_broadcast(rec_bc[:], lrec[0:1, :, :])
            nc.vector.tensor_tensor(cw[0:64, hp, :],
                                    avc[0:64, :], rec_bc[0:64, 0, :], AL.mult)
            nc.vector.tensor_tensor(cw[64:128, hp, :],
                                    avc[64:128, :], rec_bc[64:128, 1, :], AL.mult)

        # flat software pipeline over (hp, kk): AV for slot i is emitted
        # after the scores for slot i+1 plus an interleave quantum, so the
        # exp/mask latency hides behind queued PE work.
        slots = [(hp, kk) for hp in range(4) for kk in range(klim)]
        pending_av = None
        pending_hp = None
        for hp, kk in slots:
            av_f = emit_scores(hp, kk)
            done += 1
            target = done * total_mms / n_tiles
            while wi < len(work) and emitted < target:
                emitted += getattr(work[wi], "mms", 4)
                work[wi]()
                wi += 1
            if pending_av is not None:
                pending_av()
                if pending_hp is not None:
                    emit_norm(pending_hp)
                    pending_hp = None
            pending_av = av_f
            if kk == klim - 1:
                pending_hp = hp
        pending_av()
        emit_norm(pending_hp)
        while wi < len(work):
            work[wi]()
            wi += 1

    # ---- prologue: bias DMAs first (tiny; the tile framework only sees a
    # write->read dependency if the write is EMITTED before the read), then
    # Q/K for tq=0 (their DMAs gate the first matmul), then the heavy
    # persistent-tile DMAs, then V for tv=0 ----
    nc.sync.dma_start(bq_t[:], bq_d)
    nc.sync.dma_start(bk_t[:], bk_d)
    nc.sync.dma_start(sel4_t[:], sel4_d)
    for f in qk_chunks(0):
        f()
    nc.sync.dma_start(bv_t[:], bv_d)
    nc.sync.dma_start(ones_t[:], ones_d)
    nc.sync.dma_start(wv_t[:], wvT_d)
    nc.vector.tensor_copy(V65[:, :, :, 64:65],
                          ones_t[:, 0:1].to_broadcast([128, NKT, HPC, 1]))
    for f in v_chunks(0):
        f()

    # ---- windows ----
    def wo_load():
        nc.sync.dma_start(wo_t[:], woT_d)
    wo_load.mms = 0

    for qt in range(NQT):
        work = []
        if qt == 0:
            work.append(wo_load)
        if qt + 1 < NQT:
            work += qk_chunks(qt + 1)
            work += v_chunks(qt + 1)
        if qt == 2:
            work += p3_chunks(0)
        elif qt == 3:
            work += p3_chunks(1)
            work += p3_chunks(2)
        window(qt, work)
    for f in p3_chunks(NQT - 1):
        f()

    for cm in (pav_cm, psc_cm, pp_cm, py_cm, pnrm_cm, ppt_cm, pcw_cm, pq_cm,
               px_cm, pw_cm, pers_cm):
        cm.__exit__(None, None, None)


def _analyze_mask(mask):
    """Classify the mask and derive the per-q-tile k-tile structure."""
    m = np.asarray(mask)
    iota = np.arange(S)
    n = m.sum(axis=2)                     # [B, S] count of ones per row
    prefix_ok = bool((m == (iota[None, None, :] < n[..., None])).all())
    causal = prefix_ok and bool((n == iota[None, :] + 1).all())
    allones = bool((m == 1).all())

    k_needed, k_full, mixed = [], [], set()
    if allones:
        mode = "affine"   # no mask ops at all
        k_needed = [NKT] * NQT
        k_full = [NKT] * NQT
    elif causal:
        mode = "affine"
        for qt in range(NQT):
            k_needed.append(4 * qt + 4)
            k_full.append(4 * qt)
            for kk in range(4 * qt, 4 * qt + 4):
                mixed.add((qt, kk))
    else:
        mode = "dense"
        for qt in range(NQT):
            sl = m[:, qt * 512:(qt + 1) * 512, :]       # [B, 512, S]
            need = 0
            full = NKT
            for kk in range(NKT):
                blk = sl[:, :, kk * 128:(kk + 1) * 128]
                if blk.any():
                    need = kk + 1
                if not blk.all():
                    full = min(full, kk)
            need = max(need, 1)
            full = min(full, need)
            k_needed.append(need)
            k_full.append(full)
            for kk in range(full, need):
                blk = sl[:, :, kk * 128:(kk + 1) * 128]
                if not blk.all():
                    mixed.add((qt, kk))
    return mode, tuple(k_needed), tuple(k_full), frozenset(mixed)


def _prep_inputs(x_q, x_k, x_v, mask, wq, wk, wv, bq, bk, bv, wo, mode):
    """Build the 8 per-core input dicts."""
    import ml_dtypes
    f32 = np.float32
    bf16 = ml_dtypes.bfloat16
    in_maps = []
    ones1 = np.ones((128, 1), bf16)
    # sel4[p, qi, q] = 1 where q - qi*128 - p >= 0 (causal straddle masks)
    pp_ = np.arange(128)[:, None, None]
    qi_ = np.arange(4)[None, :, None]
    qq_ = np.arange(512)[None, None, :]
    sel4 = ((qq_ - qi_ * 128 - pp_) >= 0).astype(bf16)
    for core in range(NC):
        b, g = divmod(core, G)
        hs = slice(g * HPC, (g + 1) * HPC)
        im = {
            "xqT": np.ascontiguousarray(np.asarray(x_q[b], f32).T).astype(bf16),
            "xkT": np.ascontiguousarray(np.asarray(x_k[b], f32).T).astype(bf16),
            "xvT": np.ascontiguousarray(np.asarray(x_v[b], f32).T).astype(bf16),
            "ones1": ones1,
            "sel4": sel4,
        }
        for name, w in (("wqT", wq), ("wkT", wk), ("wvT", wv)):
            # [H, DH, D] group slice -> [D, EH] -> [128, NDK, EH] with d = dk*128+p
            wt = np.asarray(w[hs], f32).transpose(2, 0, 1).reshape(D, EH)
            im[name] = np.ascontiguousarray(wt.reshape(NDK, 128, EH))\
                .transpose(1, 0, 2).astype(bf16)
        for name, bb in (("bq", bq), ("bk", bk)):
            flat = np.asarray(bb[hs], f32).reshape(EH)
            im[name] = np.ascontiguousarray(flat.reshape(4, 128).T)
        im["bv"] = np.broadcast_to(np.asarray(bv[hs], f32).reshape(1, EH),
                                   (128, EH)).copy()
        woT = np.asarray(wo[:, g * EH:(g + 1) * EH], f32).T   # [EH, D]
        im["woT"] = np.ascontiguousarray(woT.reshape(4, 128, D))\
            .transpose(1, 0, 2).astype(bf16)
        if mode == "dense":
            im["maskT"] = np.ascontiguousarray(
                np.asarray(mask[b], f32).T).astype(bf16)
        in_maps.append(im)
    return in_maps


def _run(x_q, x_k, x_v, mask, wq, wk, wv, bq, bk, bv, wo, bo,
         trace=False, trace_cores=None):
    mode, k_needed, k_full, mixed = _analyze_mask(mask)
    key = (mode, k_needed, k_full, mixed)
    if key not in _cache:
        _cache[key] = _build(mode, k_needed, k_full, mixed)
    nc = _cache[key]
    in_maps = _prep_inputs(x_q, x_k, x_v, mask, wq, wk, wv, bq, bk, bv, wo, mode)
    res = run_bass_kernel_spmd(nc, in_maps, core_ids=list(range(NC)),
                               trace=trace, trace_cores=trace_cores)
    bo = np.asarray(bo, np.float32)
    out = np.empty((B, S, D), np.float32)
    for b in range(B):
        out[b] = (res.results[2 * b]["y"].astype(np.float32)
                  + res.results[2 * b + 1]["y"].astype(np.float32) + bo)
    return out, res


def kernel(x_q, x_k, x_v, mask, wq, wk, wv, bq, bk, bv, wo, bo):
    out, _ = _run(x_q, x_k, x_v, mask, wq, wk, wv, bq, bk, bv, wo, bo)
    return out


# revision 21
# speedup vs baseline: 1.2643x; 1.0302x over previous
"""MultiHeadAttention (B=4, S=2048, d_model=1024, H=16, dh=64) on 8 trn2 cores.

Sharding: core (b, g) = batch b in 0..3, head-group g in 0..1 (8 heads each).
Each core computes, for its (b, g):
  Q^T, K^T  [512, 2048] head-dim-major; V [2048, 512] token-major (+ ones col)
  transposed scores S^T = K^T_tile.T @ Q^T per (head, k-tile 128, q-tile 512)
  P = exp(S^T / 8) (no max subtraction; scores are O(1)); causal masking via
  affine_select (skip fully-masked k-tiles entirely)
  fused AV+rowsum: lhsT = [V | 1] -> psum [65, 512]; ctx normalized by 1/l via
  gpsimd partition_broadcast + one tensor_tensor (PSUM operand)
  partial output projection y_partial = ctx^T.T @ wo[:, group].T
Host sums the two groups' partials per batch and adds bo.

All matmul operands are bfloat16 (full-rate PE, half the DMA/LDWEIGHTS
bytes of fp32r; PSUM accumulation stays fp32).

Schedule: engine queues are in-order, so latency is hidden via emission
order: the AV matmuls for k-tile kk are emitted one slot late (after the
scores for kk+1 plus an interleave quantum) so the exp (scalar engine,
~1us) and affine_select (gpsimd) latencies hide behind queued PE work.
Straddle (diagonal) tiles trim the scores matmul to [moff:512] (moff <=
128; larger PSUM write offsets miscompute) and the exp to [qoff:512];
affine_select zero-fills the full tile so the AV reads no garbage.
NOTE: pool tiles must be allocated at first USE, and a tile's DMA write
must be EMITTED before any reader - the tile framework tracks deps and
pool release boundaries by emission order.
"""
import sys
sys.path.insert(0, "/opt/trn_rl_repo")

import numpy as np

import concourse.bass as bass
import concourse.mybir as mybir
import concourse.tile as tile
from concourse import bacc
from concourse.bass_utils import run_bass_kernel_spmd

F32 = mybir.dt.float32
F32R = mybir.dt.float32r
BF16 = mybir.dt.bfloat16
AF = mybir.ActivationFunctionType
AL = mybir.AluOpType

B, S, D, H, DH = 4, 2048, 1024, 16, 64
NC = 8
G = 2              # head groups (cores per batch)
HPC = H // G       # 8 heads per core
EH = HPC * DH      # 512
NQT = S // 512     # 4 q-tiles
NKT = S // 128     # 16 k-tiles
NDK = D // 128     # 8 contraction subtiles
SCALE = 1.0 / np.sqrt(DH)

_cache = {}
MM_NAMES = {"sc0": set(), "sc1": set(), "av": set(), "pj": set(), "p3": set()}


def _build(mode, k_needed, k_full, mixed):
    """Build the per-core Bass program.

    mode: "affine" (causal / prefix masks expressible as q-k>=const... strictly
          the tril case) or "dense" (per-element 0/1 mask multiply from DRAM).
    k_needed[qt]: number of leading k-tiles to compute for q-tile qt.
    k_full[qt]:   k-tiles below this index need no masking.
    mixed: set of (qt, kk) needing a mask op (affine: affine_select;
           dense: sel-tile multiply).
    """
    nc = bacc.Bacc("TRN2", target_bir_lowering=False, debug=False, num_devices=NC)

    xqT_d = nc.dram_tensor("xqT", [D, S], BF16, kind="ExternalInput").ap()
    xkT_d = nc.dram_tensor("xkT", [D, S], BF16, kind="ExternalInput").ap()
    xvT_d = nc.dram_tensor("xvT", [D, S], BF16, kind="ExternalInput").ap()
    wqT_d = nc.dram_tensor("wqT", [128, NDK, EH], BF16, kind="ExternalInput").ap()
    wkT_d = nc.dram_tensor("wkT", [128, NDK, EH], BF16, kind="ExternalInput").ap()
    wvT_d = nc.dram_tensor("wvT", [128, NDK, EH], BF16, kind="ExternalInput").ap()
    bq_d = nc.dram_tensor("bq", [128, 4], F32, kind="ExternalInput").ap()
    bk_d = nc.dram_tensor("bk", [128, 4], F32, kind="ExternalInput").ap()
    bv_d = nc.dram_tensor("bv", [128, EH], F32, kind="ExternalInput").ap()
    woT_d = nc.dram_tensor("woT", [128, 4, D], BF16, kind="ExternalInput").ap()
    ones_d = nc.dram_tensor("ones1", [128, 1], BF16, kind="ExternalInput").ap()
    sel4_d = nc.dram_tensor("sel4", [128, 4, 512], BF16, kind="ExternalInput").ap()
    if mode == "dense":
        mT_d = nc.dram_tensor("maskT", [S, S], BF16, kind="ExternalInput").ap()
        mT_v = mT_d.rearrange("(kt p) q -> p kt q", p=128)
    y_d = nc.dram_tensor("y", [S, D], BF16, kind="ExternalOutput").ap()

    xq_v = xqT_d.rearrange("(dk p) t -> p dk t", p=128)
    xk_v = xkT_d.rearrange("(dk p) t -> p dk t", p=128)
    xv_v = xvT_d.rearrange("(dk p) t -> p dk t", p=128)

    with tile.TileContext(nc) as tc:
        with nc.allow_low_precision(reason="fp32r storage has fp32 width"):
            _body(nc, tc, mode, k_needed, k_full, mixed,
                  xq_v, xk_v, xv_v, wqT_d, wkT_d, wvT_d,
                  bq_d, bk_d, bv_d, woT_d, ones_d, sel4_d,
                  mT_v if mode == "dense" else None, y_d)
    nc.compile()
    return nc


def _body(nc, tc, mode, k_needed, k_full, mixed,
          xq_v, xk_v, xv_v, wqT_d, wkT_d, wvT_d,
          bq_d, bk_d, bv_d, woT_d, ones_d, sel4_d, mT_v, y_d):
    """Interleaved schedule: Q/K/V projection chunks and output-projection
    chunks are emitted *between* attention k-tiles so the PE stays dense
    (HAM warm) and phases overlap.

    Window qt runs attention for all 4 head-pairs on q-tile qt, interleaving:
      window 0: Q/K proj for tq=1, V proj for tv=1
      window 1: tq=2, tv=2, output-proj of q-tile 0
      window 2: tq=3, tv=3, output-proj of q-tile 1
      window 3: output-proj of q-tile 2;   tail: output-proj of q-tile 3
    """
    pers_cm = tc.tile_pool(name="pers", bufs=1)
    pers = pers_cm.__enter__()
    KT = pers.tile([128, 4, S], BF16)            # [part=eh%128, et, t]
    V65 = pers.tile([128, NKT, HPC, 65], BF16)   # [t%128, t//128, h, e|1]
    wo_t = pers.tile([128, 4, D], BF16)
    wv_t = pers.tile([128, NDK, EH], BF16)
    bq_t = pers.tile([128, 4], F32)
    bk_t = pers.tile([128, 4], F32)
    bv_t = pers.tile([128, EH], F32)
    ones_t = pers.tile([128, 1], BF16)
    sel4_t = pers.tile([128, 4, 512], BF16)

    pw_cm = tc.tile_pool(name="pw", bufs=3)
    pw = pw_cm.__enter__()
    px_cm = tc.tile_pool(name="px", bufs=5)
    px = px_cm.__enter__()
    pq_cm = tc.tile_pool(name="pq", bufs=3)
    pq = pq_cm.__enter__()
    pcw_cm = tc.tile_pool(name="pcw", bufs=3)
    pcw = pcw_cm.__enter__()
    ppt_cm = tc.tile_pool(name="ppt", bufs=4)
    ppt = ppt_cm.__enter__()
    pnrm_cm = tc.tile_pool(name="pnrm", bufs=1)
    pnrm = pnrm_cm.__enter__()
    py_cm = tc.tile_pool(name="py", bufs=2)
    py = py_cm.__enter__()
    pp_cm = tc.tile_pool(name="pp", bufs=2, space="PSUM")
    pp = pp_cm.__enter__()
    psc_cm = tc.tile_pool(name="psc", bufs=2, space="PSUM")
    psc = psc_cm.__enter__()
    pav_cm = tc.tile_pool(name="pav", bufs=1, space="PSUM")
    pav = pav_cm.__enter__()

    qwin = {}    # tq -> [128, 4, 512] Q^T window tile
    ctxw = {}    # qt -> [128, 4, 512] ctx^T window tile
    state = {}   # live w/x tiles for the chunk being emitted

    # ---- chunk closures ----
    def qk_chunks(tq):
        # half-tile loads (dk 0-3 / 4-7) so bufs=3 gives cross-chunk prefetch
        def load(w_d, x_v, kind, half):
            def f():
                hs = slice(half * 4, half * 4 + 4)
                w_t = pw.tile([128, 4, EH], BF16, tag="w",
                              name=f"w_{kind}{tq}{half}")
                nc.sync.dma_start(w_t[:], w_d[:, hs, :])
                x_t = px.tile([128, 4, 512], BF16, tag="x",
                              name=f"x_{kind}{tq}{half}")
                nc.sync.dma_start(x_t[:], x_v[:, hs, tq * 512:(tq + 1) * 512])
                state[f"w{half}"], state[f"x{half}"] = w_t, x_t
                if kind == "q" and half == 0:
                    qwin[tq] = pq.tile([128, 4, 512], BF16, tag="qw",
                                       name=f"qw{tq}")
            return f

        def mmgroup(et, kind):
            def f():
                ps_t = pp.tile([128, 512], F32, tag="pj", name=f"ps_{kind}{tq}_{et}")
                for dk in range(NDK):
                    w_t = state[f"w{dk // 4}"]
                    x_t = state[f"x{dk // 4}"]
                    mi = nc.tensor.matmul(ps_t[:],
                                     w_t[:, dk % 4, et * 128:(et + 1) * 128],
                                     x_t[:, dk % 4, :],
                                     start=(dk == 0), stop=(dk == NDK - 1))
                    MM_NAMES["pj"].add(mi.ins.name)
                if kind == "q":
                    nc.vector.tensor_tensor(
                        qwin[tq][:, et, :], ps_t[:],
                        bq_t[:, et:et + 1].to_broadcast([128, 512]), AL.add)
                else:
                    nc.vector.tensor_tensor(
                        KT[:, et, tq * 512:(tq + 1) * 512], ps_t[:],
                        bk_t[:, et:et + 1].to_broadcast([128, 512]), AL.add)
            return f

        out = []
        for kind, w_d, x_v in (("q", wqT_d, xq_v), ("k", wkT_d, xk_v)):
            for half in (0, 1):
                g = load(w_d, x_v, kind, half)
                g.mms = 0
                out.append(g)
            for et in range(4):
                g = mmgroup(et, kind)
                g.mms = 8
                out.append(g)
        return out

    def v_chunks(tv):
        def load(half):
            def f():
                hs = slice(half * 4, half * 4 + 4)
                x_t = px.tile([128, 4, 512], BF16, tag="x", name=f"x_v{tv}{half}")
                nc.sync.dma_start(x_t[:], xv_v[:, hs, tv * 512:(tv + 1) * 512])
                state[f"x{half}"] = x_t
            return f

        def mmgroup(tl):
            def f():
                tt = tv * 4 + tl
                ps_t = pp.tile([128, 512], F32, tag="pj", name=f"ps_v{tt}")
                for dk in range(NDK):
                    x_t = state[f"x{dk // 4}"]
                    mi = nc.tensor.matmul(ps_t[:],
                                     x_t[:, dk % 4, tl * 128:(tl + 1) * 128],
                                     wv_t[:, dk, :],
                                     start=(dk == 0), stop=(dk == NDK - 1))
                    MM_NAMES["pj"].add(mi.ins.name)
                nc.vector.tensor_tensor(
                    V65[:, tt, :, 0:64],
                    ps_t.rearrange("p (h e) -> p h e", h=HPC),
                    bv_t.rearrange("p (h e) -> p h e", h=HPC), AL.add)
            return f

        out = []
        for half in (0, 1):
            g = load(half)
            g.mms = 0
            out.append(g)
        for tl in range(4):
            g = mmgroup(tl)
            g.mms = 8
            out.append(g)
        return out

    def p3_chunks(qt):
        p3state = {}

        def half_a(tl, mc):
            def f():
                cw = ctxw[qt]
                tt = qt * 4 + tl
                ps_t = pp.tile([128, 512], F32, tag="pj", name=f"ps_o{tt}_{mc}")
                p3state[(tl, mc)] = ps_t
                for hp in range(2):
                    mi = nc.tensor.matmul(ps_t[:],
                                          cw[:, hp, tl * 128:(tl + 1) * 128],
                                          wo_t[:, hp, mc * 512:(mc + 1) * 512],
                                          start=(hp == 0), stop=False)
                    MM_NAMES["p3"].add(mi.ins.name)
            f.mms = 2
            return f

        def half_b(tl, mc):
            def f():
                cw = ctxw[qt]
                tt = qt * 4 + tl
                ps_t = p3state.pop((tl, mc))
                for hp in range(2, 4):
                    mi = nc.tensor.matmul(ps_t[:],
                                          cw[:, hp, tl * 128:(tl + 1) * 128],
                                          wo_t[:, hp, mc * 512:(mc + 1) * 512],
                                          start=False, stop=(hp == 3))
                    MM_NAMES["p3"].add(mi.ins.name)
                y_t = py.tile([128, 512], BF16, tag="y", name=f"y{tt}_{mc}")
                nc.vector.tensor_copy(y_t[:], ps_t[:])
                nc.sync.dma_start(
                    y_d[tt * 128:(tt + 1) * 128, mc * 512:(mc + 1) * 512],
                    y_t[:])
            f.mms = 2
            return f

        out = []
        for tl in range(4):
            for mc in range(2):
                out.append(half_a(tl, mc))
                out.append(half_b(tl, mc))
        return out

    # ---- attention window with interleaved work ----
    def window(qt, work):
        klim = k_needed[qt]
        q0 = qt * 512
        cw = pcw.tile([128, 4, 512], BF16, tag="cw", name=f"cw{qt}")
        ctxw[qt] = cw
        qw = qwin[qt]
        n_tiles = 4 * klim
        total_mms = sum(getattr(f, "mms", 4) for f in work) or 1
        wi = 0
        emitted = 0
        done = 0

        av_tiles = {}   # hp -> (av0, av1)

        def emit_scores(hp, kk):
            """scores matmul pair + exp (+ mask) for (hp, kk); returns the
            closure that emits the delayed AV matmuls."""
            straddle = (qt, kk) in mixed
            qoff = max(0, kk * 128 - q0) if (straddle and mode == "affine") else 0
            # The first partial use of each of the 3 rotating p_t buffers
            # (window 0, slots 1 and 2) writes the FULL tile: every later
            # slot's stale [0:qoff) region is then a finite old exp value,
            # so the mask multiply below cannot hit NaN garbage.  Never exp
            # an s_t region the matmul did not write (stale PSUM can be NaN).
            # hp 0 of window 0 covers the first use of every rotating p_t
            # buffer (requires ppt bufs <= 4 = klim of window 0)
            force_full = qt == 0 and hp == 0
            eoff = 0 if force_full else qoff
            moff = 0 if force_full else (128 if qoff >= 128 else 0)
            s_t = psc.tile([128, 2, 512], F32, tag="sc")
            for j in range(2):
                mi = nc.tensor.matmul(
                    s_t[:, j, moff:512],
                    KT[j * 64:(j + 1) * 64, hp, kk * 128:(kk + 1) * 128],
                    qw[j * 64:(j + 1) * 64, hp, moff:512],
                    start=True, stop=True, tile_position=(j * 64, 0))
                MM_NAMES[f"sc{j}"].add(mi.ins.name)
            p_t = ppt.tile([128, 2, 512], BF16, tag="pt")
            nc.scalar.activation(p_t[:, :, eoff:512], s_t[:, :, eoff:512],
                                 AF.Exp, scale=float(SCALE))
            if straddle:
                if mode == "affine":
                    # causal mask on DVE (bf16 2x mode) with a precomputed
                    # diagonal tile; zeroes the stale [0:qoff) region too.
                    # Keeps the gpsimd queue free for partition_broadcast.
                    qi = qoff // 128
                    nc.vector.tensor_tensor(
                        p_t[:], p_t[:],
                        sel4_t[:, qi:qi + 1, :].to_broadcast([128, 2, 512]),
                        AL.mult)
                else:
                    sel_t = ppt.tile([128, 512], BF16, tag="sel")
                    nc.sync.dma_start(sel_t[:], mT_v[:, kk, q0:q0 + 512])
                    nc.vector.tensor_tensor(
                        p_t[:], p_t[:],
                        sel_t[:, None, :].to_broadcast([128, 2, 512]),
                        AL.mult)

            def emit_av():
                if kk == 0:
                    # allocate at first use: allocating earlier would put the
                    # pool release boundary (bufs=1, aliases the previous
                    # hp's accumulators) before the previous hp's final AV
                    # and norm reads - a race.
                    av_tiles[hp] = (
                        pav.tile([65, 512], F32, tag="av0",
                                 name=f"av0_{qt}_{hp}"),
                        pav.tile([65, 512], F32, tag="av1",
                                 name=f"av1_{qt}_{hp}"))
                av0, av1 = av_tiles[hp]
                for j, av in ((0, av0), (1, av1)):
                    mi = nc.tensor.matmul(av[:], V65[:, kk, 2 * hp + j, :],
                                     p_t[:, j, :],
                                     start=(kk == 0), stop=(kk == klim - 1))
                    MM_NAMES["av"].add(mi.ins.name)
            return emit_av

        def emit_norm(hp):
            # copy to SBUF first so the psum accumulators free early;
            # head1 lands at partitions 64-127 to keep TT bases aligned
            av0, av1 = av_tiles.pop(hp)
            avc = pnrm.tile([128, 512], F32, tag="avc", bufs=1)
            lin = pnrm.tile([1, 2, 512], F32, tag="lin")
            nc.vector.tensor_copy(avc[0:64, :], av0[0:64, :])
            nc.vector.tensor_copy(avc[64:128, :], av1[0:64, :])
            nc.vector.tensor_copy(lin[:, 0, :], av0[64:65, :])
            nc.vector.tensor_copy(lin[:, 1, :], av1[64:65, :])
            lrec = pnrm.tile([1, 2, 512], F32, tag="lrec")
            rec_bc = pnrm.tile([128, 2, 512], F32, tag="rbc")
            # rec_bc[0:1] doubles as reciprocal scratch (pbcast overwrites it)
            nc.vector.reciprocal_approx_accurate(lrec[:], lin[:],
                                                 rec_bc[0:1, :, :])
            nc.gpsimd.partition_broadcast(rec_bc[:], lrec[0:1, :, :])
            nc.vector.tensor_tensor(cw[0:64, hp, :],
                                    avc[0:64, :], rec_bc[0:64, 0, :], AL.mult)
            nc.vector.tensor_tensor(cw[64:128, hp, :],
                                    avc[64:128, :], rec_bc[64:128, 1, :], AL.mult)

        # flat software pipeline over (hp, kk): AV for slot i is emitted
        # after the scores for slot i+1 plus an interleave quantum, so the
        # exp/mask latency hides behind queued PE work.
        slots = [(hp, kk) for hp in range(4) for kk in range(klim)]
        pending_av = None
        pending_hp = None
        for hp, kk in slots:
            av_f = emit_scores(hp, kk)
            done += 1
            target = done * total_mms / n_tiles
            while wi < len(work) and emitted < target:
                emitted += getattr(work[wi], "mms", 4)
                work[wi]()
                wi += 1
            if pending_av is not None:
                pending_av()
                if pending_hp is not None:
                    emit_norm(pending_hp)
                    pending_hp = None
            pending_av = av_f
            if kk == klim - 1:
                pending_hp = hp
        pending_av()
        emit_norm(pending_hp)
        while wi < len(work):
            work[wi]()
            wi += 1

    # ---- prologue: bias DMAs first (tiny; the tile framework only sees a
    # write->read dependency if the write is EMITTED before the read), then
    # Q/K for tq=0 (their DMAs gate the first matmul), then the heavy
    # persistent-tile DMAs, then V for tv=0 ----
    nc.sync.dma_start(bq_t[:], bq_d)
    nc.sync.dma_start(bk_t[:], bk_d)
    nc.sync.dma_start(sel4_t[:], sel4_d)
    for f in qk_chunks(0):
        f()
    nc.sync.dma_start(bv_t[:], bv_d)
    nc.sync.dma_start(ones_t[:], ones_d)
    nc.sync.dma_start(wv_t[:], wvT_d)
    nc.vector.tensor_copy(V65[:, :, :, 64:65],
                          ones_t[:, 0:1].to_broadcast([128, NKT, HPC, 1]))
    for f in v_chunks(0):
        f()

    # ---- windows ----
    def wo_load():
        nc.sync.dma_start(wo_t[:], woT_d)
    wo_load.mms = 0

    for qt in range(NQT):
        work = []
        if qt == 0:
            work.append(wo_load)
        if qt + 1 < NQT:
            work += qk_chunks(qt + 1)
            work += v_chunks(qt + 1)
        if qt == 2:
            work += p3_chunks(0)
        elif qt == 3:
            work += p3_chunks(1)
            work += p3_chunks(2)
        window(qt, work)
    for f in p3_chunks(NQT - 1):
        f()

    for cm in (pav_cm, psc_cm, pp_cm, py_cm, pnrm_cm, ppt_cm, pcw_cm, pq_cm,
               px_cm, pw_cm, pers_cm):
        cm.__exit__(None, None, None)


def _analyze_mask(mask):
    """Classify the mask and derive the per-q-tile k-tile structure."""
    m = np.asarray(mask)
    iota = np.arange(S)
    n = m.sum(axis=2)                     # [B, S] count of ones per row
    prefix_ok = bool((m == (iota[None, None, :] < n[..., None])).all())
    causal = prefix_ok and bool((n == iota[None, :] + 1).all())
    allones = bool((m == 1).all())

    k_needed, k_full, mixed = [], [], set()
    if allones:
        mode = "affine"   # no mask ops at all
        k_needed = [NKT] * NQT
        k_full = [NKT] * NQT
    elif causal:
        mode = "affine"
        for qt in range(NQT):
            k_needed.append(4 * qt + 4)
            k_full.append(4 * qt)
            for kk in range(4 * qt, 4 * qt + 4):
                mixed.add((qt, kk))
    else:
        mode = "dense"
        for qt in range(NQT):
            sl = m[:, qt * 512:(qt + 1) * 512, :]       # [B, 512, S]
            need = 0
            full = NKT
            for kk in range(NKT):
                blk = sl[:, :, kk * 128:(kk + 1) * 128]
                if blk.any():
                    need = kk + 1
                if not blk.all():
                    full = min(full, kk)
            need = max(need, 1)
            full = min(full, need)
            k_needed.append(need)
            k_full.append(full)
            for kk in range(full, need):
                blk = sl[:, :, kk * 128:(kk + 1) * 128]
                if not blk.all():
                    mixed.add((qt, kk))
    return mode, tuple(k_needed), tuple(k_full), frozenset(mixed)


def _prep_inputs(x_q, x_k, x_v, mask, wq, wk, wv, bq, bk, bv, wo, mode):
    """Build the 8 per-core input dicts."""
    import ml_dtypes
    f32 = np.float32
    bf16 = ml_dtypes.bfloat16
    in_maps = []
    ones1 = np.ones((128, 1), bf16)
    # sel4[p, qi, q] = 1 where q - qi*128 - p >= 0 (causal straddle masks)
    pp_ = np.arange(128)[:, None, None]
    qi_ = np.arange(4)[None, :, None]
    qq_ = np.arange(512)[None, None, :]
    sel4 = ((qq_ - qi_ * 128 - pp_) >= 0).astype(bf16)
    for core in range(NC):
        b, g = divmod(core, G)
        hs = slice(g * HPC, (g + 1) * HPC)
        im = {
            "xqT": np.ascontiguousarray(np.asarray(x_q[b], f32).T).astype(bf16),
            "xkT": np.ascontiguousarray(np.asarray(x_k[b], f32).T).astype(bf16),
            "xvT": np.ascontiguousarray(np.asarray(x_v[b], f32).T).astype(bf16),
            "ones1": ones1,
            "sel4": sel4,
        }
        for name, w in (("wqT", wq), ("wkT", wk), ("wvT", wv)):
            # [H, DH, D] group slice -> [D, EH] -> [128, NDK, EH] with d = dk*128+p
            wt = np.asarray(w[hs], f32).transpose(2, 0, 1).reshape(D, EH)
            im[name] = np.ascontiguousarray(wt.reshape(NDK, 128, EH))\
                .transpose(1, 0, 2).astype(bf16)
        for name, bb in (("bq", bq), ("bk", bk)):
            flat = np.asarray(bb[hs], f32).reshape(EH)
            im[name] = np.ascontiguousarray(flat.reshape(4, 128).T)
        im["bv"] = np.broadcast_to(np.asarray(bv[hs], f32).reshape(1, EH),
                                   (128, EH)).copy()
        woT = np.asarray(wo[:, g * EH:(g + 1) * EH], f32).T   # [EH, D]
        im["woT"] = np.ascontiguousarray(woT.reshape(4, 128, D))\
            .transpose(1, 0, 2).astype(bf16)
        if mode == "dense":
            im["maskT"] = np.ascontiguousarray(
                np.asarray(mask[b], f32).T).astype(bf16)
        in_maps.append(im)
    return in_maps


def _run(x_q, x_k, x_v, mask, wq, wk, wv, bq, bk, bv, wo, bo,
         trace=False, trace_cores=None):
    mode, k_needed, k_full, mixed = _analyze_mask(mask)
    key = (mode, k_needed, k_full, mixed)
    if key not in _cache:
        _cache[key] = _build(mode, k_needed, k_full, mixed)
    nc = _cache[key]
    in_maps = _prep_inputs(x_q, x_k, x_v, mask, wq, wk, wv, bq, bk, bv, wo, mode)
    res = run_bass_kernel_spmd(nc, in_maps, core_ids=list(range(NC)),
                               trace=trace, trace_cores=trace_cores)
    bo = np.asarray(bo, np.float32)
    out = np.empty((B, S, D), np.float32)
    for b in range(B):
        out[b] = (res.results[2 * b]["y"].astype(np.float32)
                  + res.results[2 * b + 1]["y"].astype(np.float32) + bo)
    return out, res


def kernel(x_q, x_k, x_v, mask, wq, wk, wv, bq, bk, bv, wo, bo):
    out, _ = _run(x_q, x_k, x_v, mask, wq, wk, wv, bq, bk, bv, wo, bo)
    return out


# revision 22
# speedup vs baseline: 1.2787x; 1.0113x over previous
"""MultiHeadAttention (B=4, S=2048, d_model=1024, H=16, dh=64) on 8 trn2 cores.

Sharding: core (b, g) = batch b in 0..3, head-group g in 0..1 (8 heads each).
Each core computes, for its (b, g):
  Q^T, K^T  [512, 2048] head-dim-major; V [2048, 512] token-major (+ ones col)
  transposed scores S^T = K^T_tile.T @ Q^T per (head, k-tile 128, q-tile 512)
  P = exp(S^T / 8) (no max subtraction; scores are O(1)); causal masking via
  affine_select (skip fully-masked k-tiles entirely)
  fused AV+rowsum: lhsT = [V | 1] -> psum [65, 512]; ctx normalized by 1/l via
  gpsimd partition_broadcast + one tensor_tensor (PSUM operand)
  partial output projection y_partial = ctx^T.T @ wo[:, group].T
Host sums the two groups' partials per batch and adds bo.

All matmul operands are bfloat16 (full-rate PE, half the DMA/LDWEIGHTS
bytes of fp32r; PSUM accumulation stays fp32).

Schedule: engine queues are in-order, so latency is hidden via emission
order: the AV matmuls for k-tile kk are emitted one slot late (after the
scores for kk+1 plus an interleave quantum) so the exp (scalar engine,
~1us) and affine_select (gpsimd) latencies hide behind queued PE work.
Straddle (diagonal) tiles trim the scores matmul to [moff:512] (moff <=
128; larger PSUM write offsets miscompute) and the exp to [qoff:512];
affine_select zero-fills the full tile so the AV reads no garbage.
NOTE: pool tiles must be allocated at first USE, and a tile's DMA write
must be EMITTED before any reader - the tile framework tracks deps and
pool release boundaries by emission order.
"""
import sys
sys.path.insert(0, "/opt/trn_rl_repo")

import numpy as np

import concourse.bass as bass
import concourse.mybir as mybir
import concourse.tile as tile
from concourse import bacc
from concourse.bass_utils import run_bass_kernel_spmd

F32 = mybir.dt.float32
F32R = mybir.dt.float32r
BF16 = mybir.dt.bfloat16
AF = mybir.ActivationFunctionType
AL = mybir.AluOpType

B, S, D, H, DH = 4, 2048, 1024, 16, 64
NC = 8
G = 2              # head groups (cores per batch)
HPC = H // G       # 8 heads per core
EH = HPC * DH      # 512
NQT = S // 512     # 4 q-tiles
NKT = S // 128     # 16 k-tiles
NDK = D // 128     # 8 contraction subtiles
SCALE = 1.0 / np.sqrt(DH)

_cache = {}
MM_NAMES = {"sc0": set(), "sc1": set(), "av": set(), "pj": set(), "p3": set()}


def _build(mode, k_needed, k_full, mixed):
    """Build the per-core Bass program.

    mode: "affine" (causal / prefix masks expressible as q-k>=const... strictly
          the tril case) or "dense" (per-element 0/1 mask multiply from DRAM).
    k_needed[qt]: number of leading k-tiles to compute for q-tile qt.
    k_full[qt]:   k-tiles below this index need no masking.
    mixed: set of (qt, kk) needing a mask op (affine: affine_select;
           dense: sel-tile multiply).
    """
    nc = bacc.Bacc("TRN2", target_bir_lowering=False, debug=False, num_devices=NC)

    xqT_d = nc.dram_tensor("xqT", [D, S], BF16, kind="ExternalInput").ap()
    xkT_d = nc.dram_tensor("xkT", [D, S], BF16, kind="ExternalInput").ap()
    xvT_d = nc.dram_tensor("xvT", [D, S], BF16, kind="ExternalInput").ap()
    wqT_d = nc.dram_tensor("wqT", [128, NDK, EH], BF16, kind="ExternalInput").ap()
    wkT_d = nc.dram_tensor("wkT", [128, NDK, EH], BF16, kind="ExternalInput").ap()
    wvT_d = nc.dram_tensor("wvT", [128, NDK, EH], BF16, kind="ExternalInput").ap()
    bq_d = nc.dram_tensor("bq", [128, 4], F32, kind="ExternalInput").ap()
    bk_d = nc.dram_tensor("bk", [128, 4], F32, kind="ExternalInput").ap()
    bv_d = nc.dram_tensor("bv", [128, EH], F32, kind="ExternalInput").ap()
    woT_d = nc.dram_tensor("woT", [128, 4, D], BF16, kind="ExternalInput").ap()
    ones_d = nc.dram_tensor("ones1", [128, 1], BF16, kind="ExternalInput").ap()
    sel4_d = nc.dram_tensor("sel4", [128, 4, 512], BF16, kind="ExternalInput").ap()
    if mode == "dense":
        mT_d = nc.dram_tensor("maskT", [S, S], BF16, kind="ExternalInput").ap()
        mT_v = mT_d.rearrange("(kt p) q -> p kt q", p=128)
    y_d = nc.dram_tensor("y", [S, D], BF16, kind="ExternalOutput").ap()

    xq_v = xqT_d.rearrange("(dk p) t -> p dk t", p=128)
    xk_v = xkT_d.rearrange("(dk p) t -> p dk t", p=128)
    xv_v = xvT_d.rearrange("(dk p) t -> p dk t", p=128)

    with tile.TileContext(nc) as tc:
        with nc.allow_low_precision(reason="fp32r storage has fp32 width"):
            _body(nc, tc, mode, k_needed, k_full, mixed,
                  xq_v, xk_v, xv_v, wqT_d, wkT_d, wvT_d,
                  bq_d, bk_d, bv_d, woT_d, ones_d, sel4_d,
                  mT_v if mode == "dense" else None, y_d)
    nc.compile()
    return nc


def _body(nc, tc, mode, k_needed, k_full, mixed,
          xq_v, xk_v, xv_v, wqT_d, wkT_d, wvT_d,
          bq_d, bk_d, bv_d, woT_d, ones_d, sel4_d, mT_v, y_d):
    """Interleaved schedule: Q/K/V projection chunks and output-projection
    chunks are emitted *between* attention k-tiles so the PE stays dense
    (HAM warm) and phases overlap.

    Window qt runs attention for all 4 head-pairs on q-tile qt, interleaving:
      window 0: Q/K proj for tq=1, V proj for tv=1
      window 1: tq=2, tv=2, output-proj of q-tile 0
      window 2: tq=3, tv=3, output-proj of q-tile 1
      window 3: output-proj of q-tile 2;   tail: output-proj of q-tile 3
    """
    pers_cm = tc.tile_pool(name="pers", bufs=1)
    pers = pers_cm.__enter__()
    KT = pers.tile([128, 4, S], BF16)            # [part=eh%128, et, t]
    V65 = pers.tile([128, NKT, HPC, 65], BF16)   # [t%128, t//128, h, e|1]
    wo_t = pers.tile([128, 4, D], BF16)
    wv_t = pers.tile([128, NDK, EH], BF16)
    bq_t = pers.tile([128, 4], F32)
    bk_t = pers.tile([128, 4], F32)
    bv_t = pers.tile([128, EH], F32)
    ones_t = pers.tile([128, 1], BF16)
    sel4_t = pers.tile([128, 4, 512], BF16)

    pw_cm = tc.tile_pool(name="pw", bufs=3)
    pw = pw_cm.__enter__()
    px_cm = tc.tile_pool(name="px", bufs=5)
    px = px_cm.__enter__()
    pq_cm = tc.tile_pool(name="pq", bufs=3)
    pq = pq_cm.__enter__()
    pcw_cm = tc.tile_pool(name="pcw", bufs=3)
    pcw = pcw_cm.__enter__()
    ppt_cm = tc.tile_pool(name="ppt", bufs=4)
    ppt = ppt_cm.__enter__()
    pnrm_cm = tc.tile_pool(name="pnrm", bufs=1)
    pnrm = pnrm_cm.__enter__()
    py_cm = tc.tile_pool(name="py", bufs=2)
    py = py_cm.__enter__()
    pp_cm = tc.tile_pool(name="pp", bufs=2, space="PSUM")
    pp = pp_cm.__enter__()
    psc_cm = tc.tile_pool(name="psc", bufs=2, space="PSUM")
    psc = psc_cm.__enter__()
    pav_cm = tc.tile_pool(name="pav", bufs=1, space="PSUM")
    pav = pav_cm.__enter__()

    qwin = {}    # tq -> [128, 4, 512] Q^T window tile
    ctxw = {}    # qt -> [128, 4, 512] ctx^T window tile
    state = {}   # live w/x tiles for the chunk being emitted

    # ---- chunk closures ----
    def qk_chunks(tq):
        # half-tile loads (dk 0-3 / 4-7) so bufs=3 gives cross-chunk prefetch
        def load(w_d, x_v, kind, half):
            def f():
                hs = slice(half * 4, half * 4 + 4)
                w_t = pw.tile([128, 4, EH], BF16, tag="w",
                              name=f"w_{kind}{tq}{half}")
                nc.sync.dma_start(w_t[:], w_d[:, hs, :])
                x_t = px.tile([128, 4, 512], BF16, tag="x",
                              name=f"x_{kind}{tq}{half}")
                nc.sync.dma_start(x_t[:], x_v[:, hs, tq * 512:(tq + 1) * 512])
                state[f"w{half}"], state[f"x{half}"] = w_t, x_t
                if kind == "q" and half == 0:
                    qwin[tq] = pq.tile([128, 4, 512], BF16, tag="qw",
                                       name=f"qw{tq}")
            return f

        def mmgroup(et, kind):
            def f():
                ps_t = pp.tile([128, 512], F32, tag="pj", name=f"ps_{kind}{tq}_{et}")
                for dk in range(NDK):
                    w_t = state[f"w{dk // 4}"]
                    x_t = state[f"x{dk // 4}"]
                    mi = nc.tensor.matmul(ps_t[:],
                                     w_t[:, dk % 4, et * 128:(et + 1) * 128],
                                     x_t[:, dk % 4, :],
                                     start=(dk == 0), stop=(dk == NDK - 1))
                    MM_NAMES["pj"].add(mi.ins.name)
                if kind == "q":
                    nc.vector.tensor_tensor(
                        qwin[tq][:, et, :], ps_t[:],
                        bq_t[:, et:et + 1].to_broadcast([128, 512]), AL.add)
                else:
                    nc.vector.tensor_tensor(
                        KT[:, et, tq * 512:(tq + 1) * 512], ps_t[:],
                        bk_t[:, et:et + 1].to_broadcast([128, 512]), AL.add)
            return f

        out = []
        for kind, w_d, x_v in (("q", wqT_d, xq_v), ("k", wkT_d, xk_v)):
            for half in (0, 1):
                g = load(w_d, x_v, kind, half)
                g.mms = 0
                out.append(g)
            for et in range(4):
                g = mmgroup(et, kind)
                g.mms = 8
                out.append(g)
        return out

    def v_chunks(tv):
        def load(half):
            def f():
                hs = slice(half * 4, half * 4 + 4)
                x_t = px.tile([128, 4, 512], BF16, tag="x", name=f"x_v{tv}{half}")
                nc.sync.dma_start(x_t[:], xv_v[:, hs, tv * 512:(tv + 1) * 512])
                state[f"x{half}"] = x_t
            return f

        def mmgroup(tl):
            def f():
                tt = tv * 4 + tl
                ps_t = pp.tile([128, 512], F32, tag="pj", name=f"ps_v{tt}")
                for dk in range(NDK):
                    x_t = state[f"x{dk // 4}"]
                    mi = nc.tensor.matmul(ps_t[:],
                                     x_t[:, dk % 4, tl * 128:(tl + 1) * 128],
                                     wv_t[:, dk, :],
                                     start=(dk == 0), stop=(dk == NDK - 1))
                    MM_NAMES["pj"].add(mi.ins.name)
                nc.vector.tensor_tensor(
                    V65[:, tt, :, 0:64],
                    ps_t.rearrange("p (h e) -> p h e", h=HPC),
                    bv_t.rearrange("p (h e) -> p h e", h=HPC), AL.add)
            return f

        out = []
        for half in (0, 1):
            g = load(half)
            g.mms = 0
            out.append(g)
        for tl in range(4):
            g = mmgroup(tl)
            g.mms = 8
            out.append(g)
        return out

    def p3_chunks(qt):
        p3state = {}

        def half_a(tl, mc):
            def f():
                cw = ctxw[qt]
                tt = qt * 4 + tl
                ps_t = pp.tile([128, 512], F32, tag="pj", name=f"ps_o{tt}_{mc}")
                p3state[(tl, mc)] = ps_t
                for hp in range(2):
                    mi = nc.tensor.matmul(ps_t[:],
                                          cw[:, hp, tl * 128:(tl + 1) * 128],
                                          wo_t[:, hp, mc * 512:(mc + 1) * 512],
                                          start=(hp == 0), stop=False)
                    MM_NAMES["p3"].add(mi.ins.name)
            f.mms = 2
            return f

        def half_b(tl, mc):
            def f():
                cw = ctxw[qt]
                tt = qt * 4 + tl
                ps_t = p3state.pop((tl, mc))
                for hp in range(2, 4):
                    mi = nc.tensor.matmul(ps_t[:],
                                          cw[:, hp, tl * 128:(tl + 1) * 128],
                                          wo_t[:, hp, mc * 512:(mc + 1) * 512],
                                          start=False, stop=(hp == 3))
                    MM_NAMES["p3"].add(mi.ins.name)
                y_t = py.tile([128, 512], BF16, tag="y", name=f"y{tt}_{mc}")
                nc.vector.tensor_copy(y_t[:], ps_t[:])
                nc.sync.dma_start(
                    y_d[tt * 128:(tt + 1) * 128, mc * 512:(mc + 1) * 512],
                    y_t[:])
            f.mms = 2
            return f

        out = []
        for tl in range(4):
            for mc in range(2):
                out.append(half_a(tl, mc))
                out.append(half_b(tl, mc))
        return out

    # ---- attention window with interleaved work ----
    def window(qt, work):
        klim = k_needed[qt]
        q0 = qt * 512
        cw = pcw.tile([128, 4, 512], BF16, tag="cw", name=f"cw{qt}")
        ctxw[qt] = cw
        qw = qwin[qt]
        n_tiles = 4 * klim
        total_mms = sum(getattr(f, "mms", 4) for f in work) or 1
        wi = 0
        emitted = 0
        done = 0

        av_tiles = {}   # hp -> (av0, av1)

        def emit_scores(hp, kk):
            """scores matmul pair + exp (+ mask) for (hp, kk); returns the
            closure that emits the delayed AV matmuls."""
            straddle = (qt, kk) in mixed
            qoff = max(0, kk * 128 - q0) if (straddle and mode == "affine") else 0
            # The first partial use of each of the 3 rotating p_t buffers
            # (window 0, slots 1 and 2) writes the FULL tile: every later
            # slot's stale [0:qoff) region is then a finite old exp value,
            # so the mask multiply below cannot hit NaN garbage.  Never exp
            # an s_t region the matmul did not write (stale PSUM can be NaN).
            # hp 0 of window 0 covers the first use of every rotating p_t
            # buffer (requires ppt bufs <= 4 = klim of window 0)
            force_full = qt == 0 and hp == 0
            eoff = 0 if force_full else qoff
            moff = 0 if force_full else (128 if qoff >= 128 else 0)
            s_t = psc.tile([128, 2, 512], F32, tag="sc")
            for j in range(2):
                mi = nc.tensor.matmul(
                    s_t[:, j, moff:512],
                    KT[j * 64:(j + 1) * 64, hp, kk * 128:(kk + 1) * 128],
                    qw[j * 64:(j + 1) * 64, hp, moff:512],
                    start=True, stop=True, tile_position=(j * 64, 0))
                MM_NAMES[f"sc{j}"].add(mi.ins.name)
            p_t = ppt.tile([128, 2, 512], BF16, tag="pt")
            nc.scalar.activation(p_t[:, :, eoff:512], s_t[:, :, eoff:512],
                                 AF.Exp, scale=float(SCALE))
            if straddle:
                if mode == "affine":
                    # causal mask on DVE (bf16 2x mode) with a precomputed
                    # diagonal tile; zeroes the stale [0:qoff) region too.
                    # Keeps the gpsimd queue free for partition_broadcast.
                    qi = qoff // 128
                    nc.vector.tensor_tensor(
                        p_t[:], p_t[:],
                        sel4_t[:, qi:qi + 1, :].to_broadcast([128, 2, 512]),
                        AL.mult)
                else:
                    sel_t = ppt.tile([128, 512], BF16, tag="sel")
                    nc.sync.dma_start(sel_t[:], mT_v[:, kk, q0:q0 + 512])
                    nc.vector.tensor_tensor(
                        p_t[:], p_t[:],
                        sel_t[:, None, :].to_broadcast([128, 2, 512]),
                        AL.mult)

            def emit_av():
                if kk == 0:
                    # allocate at first use: allocating earlier would put the
                    # pool release boundary (bufs=1, aliases the previous
                    # hp's accumulators) before the previous hp's final AV
                    # and norm reads - a race.
                    av_tiles[hp] = (
                        pav.tile([65, 512], F32, tag="av0",
                                 name=f"av0_{qt}_{hp}"),
                        pav.tile([65, 512], F32, tag="av1",
                                 name=f"av1_{qt}_{hp}"))
                av0, av1 = av_tiles[hp]
                for j, av in ((0, av0), (1, av1)):
                    mi = nc.tensor.matmul(av[:], V65[:, kk, 2 * hp + j, :],
                                     p_t[:, j, :],
                                     start=(kk == 0), stop=(kk == klim - 1))
                    MM_NAMES["av"].add(mi.ins.name)
            return emit_av

        def emit_norm(hp, last=False):
            av0, av1 = av_tiles.pop(hp)
            if last:
                # kernel is ending: no need to free the av PSUM early, so
                # skip the avc staging copies and let the cw multiplies read
                # PSUM directly (tensor_tensor with one PSUM operand).  The
                # reciprocal still reads SBUF (its custom-DVE ops cannot
                # take PSUM operands); rec_bc[0:64] equals [64:128] after
                # the partition broadcast, keeping operand bases aligned.
                lin = pnrm.tile([1, 2, 512], F32, tag="lin", name="linL")
                nc.vector.tensor_copy(lin[:, 0, :], av0[64:65, :])
                nc.vector.tensor_copy(lin[:, 1, :], av1[64:65, :])
                lrec = pnrm.tile([1, 2, 512], F32, tag="lrec", name="lrecL")
                rec_bc = pnrm.tile([128, 2, 512], F32, tag="rbc", name="rbcL")
                nc.vector.reciprocal_approx_accurate(lrec[:], lin[:],
                                                     rec_bc[0:1, :, :])
                nc.gpsimd.partition_broadcast(rec_bc[:], lrec[0:1, :, :])
                nc.vector.tensor_tensor(cw[0:64, hp, :], av0[0:64, :],
                                        rec_bc[0:64, 0, :], AL.mult)
                nc.vector.tensor_tensor(cw[64:128, hp, :], av1[0:64, :],
                                        rec_bc[0:64, 1, :], AL.mult)
                return
            # copy to SBUF first so the psum accumulators free early;
            # head1 lands at partitions 64-127 to keep TT bases aligned
            avc = pnrm.tile([128, 512], F32, tag="avc", bufs=1)
            lin = pnrm.tile([1, 2, 512], F32, tag="lin")
            nc.vector.tensor_copy(avc[0:64, :], av0[0:64, :])
            nc.vector.tensor_copy(avc[64:128, :], av1[0:64, :])
            nc.vector.tensor_copy(lin[:, 0, :], av0[64:65, :])
            nc.vector.tensor_copy(lin[:, 1, :], av1[64:65, :])
            lrec = pnrm.tile([1, 2, 512], F32, tag="lrec")
            rec_bc = pnrm.tile([128, 2, 512], F32, tag="rbc")
            # rec_bc[0:1] doubles as reciprocal scratch (pbcast overwrites it)
            nc.vector.reciprocal_approx_accurate(lrec[:], lin[:],
                                                 rec_bc[0:1, :, :])
            nc.gpsimd.partition_broadcast(rec_bc[:], lrec[0:1, :, :])
            nc.vector.tensor_tensor(cw[0:64, hp, :],
                                    avc[0:64, :], rec_bc[0:64, 0, :], AL.mult)
            nc.vector.tensor_tensor(cw[64:128, hp, :],
                                    avc[64:128, :], rec_bc[64:128, 1, :], AL.mult)

        # flat software pipeline over (hp, kk): AV for slot i is emitted
        # after the scores for slot i+1 plus an interleave quantum, so the
        # exp/mask latency hides behind queued PE work.
        slots = [(hp, kk) for hp in range(4) for kk in range(klim)]
        pending_av = None
        pending_hp = None
        for hp, kk in slots:
            av_f = emit_scores(hp, kk)
            done += 1
            target = done * total_mms / n_tiles
            while wi < len(work) and emitted < target:
                emitted += getattr(work[wi], "mms", 4)
                work[wi]()
                wi += 1
            if pending_av is not None:
                pending_av()
                if pending_hp is not None:
                    emit_norm(pending_hp)
                    pending_hp = None
            pending_av = av_f
            if kk == klim - 1:
                pending_hp = hp
        pending_av()
        emit_norm(pending_hp, last=(qt == NQT - 1))
        while wi < len(work):
            work[wi]()
            wi += 1

    # ---- prologue: bias DMAs first (tiny; the tile framework only sees a
    # write->read dependency if the write is EMITTED before the read), then
    # Q/K for tq=0 (their DMAs gate the first matmul), then the heavy
    # persistent-tile DMAs, then V for tv=0 ----
    nc.sync.dma_start(bq_t[:], bq_d)
    nc.sync.dma_start(bk_t[:], bk_d)
    nc.sync.dma_start(sel4_t[:], sel4_d)
    for f in qk_chunks(0):
        f()
    nc.sync.dma_start(bv_t[:], bv_d)
    nc.sync.dma_start(ones_t[:], ones_d)
    nc.sync.dma_start(wv_t[:], wvT_d)
    nc.vector.tensor_copy(V65[:, :, :, 64:65],
                          ones_t[:, 0:1].to_broadcast([128, NKT, HPC, 1]))
    for f in v_chunks(0):
        f()

    # ---- windows ----
    def wo_load():
        nc.sync.dma_start(wo_t[:], woT_d)
    wo_load.mms = 0

    for qt in range(NQT):
        work = []
        if qt == 0:
            work.append(wo_load)
        if qt + 1 < NQT:
            work += qk_chunks(qt + 1)
            work += v_chunks(qt + 1)
        if qt == 2:
            work += p3_chunks(0)
        elif qt == 3:
            work += p3_chunks(1)
            work += p3_chunks(2)
        window(qt, work)
    for f in p3_chunks(NQT - 1):
        f()

    for cm in (pav_cm, psc_cm, pp_cm, py_cm, pnrm_cm, ppt_cm, pcw_cm, pq_cm,
               px_cm, pw_cm, pers_cm):
        cm.__exit__(None, None, None)


def _analyze_mask(mask):
    """Classify the mask and derive the per-q-tile k-tile structure."""
    m = np.asarray(mask)
    iota = np.arange(S)
    n = m.sum(axis=2)                     # [B, S] count of ones per row
    prefix_ok = bool((m == (iota[None, None, :] < n[..., None])).all())
    causal = prefix_ok and bool((n == iota[None, :] + 1).all())
    allones = bool((m == 1).all())

    k_needed, k_full, mixed = [], [], set()
    if allones:
        mode = "affine"   # no mask ops at all
        k_needed = [NKT] * NQT
        k_full = [NKT] * NQT
    elif causal:
        mode = "affine"
        for qt in range(NQT):
            k_needed.append(4 * qt + 4)
            k_full.append(4 * qt)
            for kk in range(4 * qt, 4 * qt + 4):
                mixed.add((qt, kk))
    else:
        mode = "dense"
        for qt in range(NQT):
            sl = m[:, qt * 512:(qt + 1) * 512, :]       # [B, 512, S]
            need = 0
            full = NKT
            for kk in range(NKT):
                blk = sl[:, :, kk * 128:(kk + 1) * 128]
                if blk.any():
                    need = kk + 1
                if not blk.all():
                    full = min(full, kk)
            need = max(need, 1)
            full = min(full, need)
            k_needed.append(need)
            k_full.append(full)
            for kk in range(full, need):
                blk = sl[:, :, kk * 128:(kk + 1) * 128]
                if not blk.all():
                    mixed.add((qt, kk))
    return mode, tuple(k_needed), tuple(k_full), frozenset(mixed)


def _prep_inputs(x_q, x_k, x_v, mask, wq, wk, wv, bq, bk, bv, wo, mode):
    """Build the 8 per-core input dicts."""
    import ml_dtypes
    f32 = np.float32
    bf16 = ml_dtypes.bfloat16
    in_maps = []
    ones1 = np.ones((128, 1), bf16)
    # sel4[p, qi, q] = 1 where q - qi*128 - p >= 0 (causal straddle masks)
    pp_ = np.arange(128)[:, None, None]
    qi_ = np.arange(4)[None, :, None]
    qq_ = np.arange(512)[None, None, :]
    sel4 = ((qq_ - qi_ * 128 - pp_) >= 0).astype(bf16)
    for core in range(NC):
        b, g = divmod(core, G)
        hs = slice(g * HPC, (g + 1) * HPC)
        im = {
            "xqT": np.ascontiguousarray(np.asarray(x_q[b], f32).T).astype(bf16),
            "xkT": np.ascontiguousarray(np.asarray(x_k[b], f32).T).astype(bf16),
            "xvT": np.ascontiguousarray(np.asarray(x_v[b], f32).T).astype(bf16),
            "ones1": ones1,
            "sel4": sel4,
        }
        for name, w in (("wqT", wq), ("wkT", wk), ("wvT", wv)):
            # [H, DH, D] group slice -> [D, EH] -> [128, NDK, EH] with d = dk*128+p
            wt = np.asarray(w[hs], f32).transpose(2, 0, 1).reshape(D, EH)
            im[name] = np.ascontiguousarray(wt.reshape(NDK, 128, EH))\
                .transpose(1, 0, 2).astype(bf16)
        for name, bb in (("bq", bq), ("bk", bk)):
            flat = np.asarray(bb[hs], f32).reshape(EH)
            im[name] = np.ascontiguousarray(flat.reshape(4, 128).T)
        im["bv"] = np.broadcast_to(np.asarray(bv[hs], f32).reshape(1, EH),
                                   (128, EH)).copy()
        woT = np.asarray(wo[:, g * EH:(g + 1) * EH], f32).T   # [EH, D]
        im["woT"] = np.ascontiguousarray(woT.reshape(4, 128, D))\
            .transpose(1, 0, 2).astype(bf16)
        if mode == "dense":
            im["maskT"] = np.ascontiguousarray(
                np.asarray(mask[b], f32).T).astype(bf16)
        in_maps.append(im)
    return in_maps


def _run(x_q, x_k, x_v, mask, wq, wk, wv, bq, bk, bv, wo, bo,
         trace=False, trace_cores=None):
    mode, k_needed, k_full, mixed = _analyze_mask(mask)
    key = (mode, k_needed, k_full, mixed)
    if key not in _cache:
        _cache[key] = _build(mode, k_needed, k_full, mixed)
    nc = _cache[key]
    in_maps = _prep_inputs(x_q, x_k, x_v, mask, wq, wk, wv, bq, bk, bv, wo, mode)
    res = run_bass_kernel_spmd(nc, in_maps, core_ids=list(range(NC)),
                               trace=trace, trace_cores=trace_cores)
    bo = np.asarray(bo, np.float32)
    out = np.empty((B, S, D), np.float32)
    for b in range(B):
        out[b] = (res.results[2 * b]["y"].astype(np.float32)
                  + res.results[2 * b + 1]["y"].astype(np.float32) + bo)
    return out, res


def kernel(x_q, x_k, x_v, mask, wq, wk, wv, bq, bk, bv, wo, bo):
    out, _ = _run(x_q, x_k, x_v, mask, wq, wk, wv, bq, bk, bv, wo, bo)
    return out
